# revision 30
# baseline (speedup 1.0000x reference)
"""Trainium2 Bass kernel for CausalSelfAttention (B=1, T=2048, C=4096,
32 heads / 8 query groups / head_size 128, full-dim RoPE, GQA).

Sharding: tensor-parallel over the 8 query groups. Core g owns w_attn rows
[g*768:(g+1)*768] (4 q heads + 1 k + 1 v) and w_proj columns
[g*512:(g+1)*512]; x is replicated. Each core returns a partial projection
output [2048, 4096] (bf16); the host sums the 8 partials (the all-reduce).

v1: bf16 datapath (same PE cost as float32r but half DMA/SBUF, faster DVE),
softmax denominator on the GpSimd/Pool engine instead of PE matmuls, exact
causal narrowing on diagonal tiles, batched DMAs, bf16 staged output.
"""

import os
import sys

for _p in ("/opt/trn_rl_repo", "/root/.axon_site/_ro/trn_rl_repo"):
    if os.path.isdir(_p) and _p not in sys.path:
        sys.path.insert(0, _p)

import numpy as np
import ml_dtypes

import concourse.bass as bass
import concourse.mybir as mybir
import concourse.tile as tile
from concourse import bacc, bass_utils

N_CORES = 8
T = 2048
C = 4096
HS = 128
N_HEAD = 32
G = 8                      # query groups == cores
QPK = 4                    # q heads per group
NCOMP = QPK + 2            # q0..q3, k, v
RG = NCOMP * HS            # 768 w_attn rows per group
OG = QPK * HS              # 512 proj-input cols per group
NT = T // 512              # 4 blocks of 512 along t
NC = C // 128              # 32 contraction chunks
NQ = C // 512              # 8 contraction quads
SCALE = 1.0 / np.sqrt(float(HS))

F32 = mybir.dt.float32
F32R = mybir.dt.float32r
BF16 = mybir.dt.bfloat16
FP8 = mybir.dt.float8e4
NPBF16 = ml_dtypes.bfloat16
NP8 = ml_dtypes.float8_e4m3
DR = mybir.MatmulPerfMode.DoubleRow
NC2 = C // 256             # 16 double-row pair chunks
WSCALE = 64.0              # weight pre-scale so fp8 operands are ~unit sigma


def _build_program():
    nc = bacc.Bacc(trn_type="TRN2", target_bir_lowering=False, debug=False,
                   num_devices=N_CORES)

    d_x = nc.dram_tensor("x8", [2, C, T], FP8, kind="ExternalInput").ap()
    d_wa = nc.dram_tensor("wa8", [2, C, RG], FP8, kind="ExternalInput").ap()
    d_wp = nc.dram_tensor("wp8", [2, OG, C], FP8, kind="ExternalInput").ap()
    d_cos = nc.dram_tensor("cost", [HS, T], BF16, kind="ExternalInput").ap()
    d_sin = nc.dram_tensor("sint", [HS, T], BF16, kind="ExternalInput").ap()
    d_tri = nc.dram_tensor("tri", [128, 128], BF16, kind="ExternalInput").ap()
    d_perm = nc.dram_tensor("perm", [128, 128], BF16, kind="ExternalInput").ap()
    d_idn = nc.dram_tensor("idn", [128, 128], BF16, kind="ExternalInput").ap()
    d_ones = nc.dram_tensor("ones", [128, 128], BF16, kind="ExternalInput").ap()
    d_out = nc.dram_tensor("out", [T, C], BF16, kind="ExternalOutput").ap()

    with tile.TileContext(nc) as tc:
        with tc.tile_pool(name="glob", bufs=1) as glob:
            # roped q0..q3 / k, one tile per (comp, t-block): [hs=128, 512]
            QQ = [[glob.tile([128, 512], BF16, name=f"qq{j}_{tb}",
                             tag=f"qq{j}_{tb}")
                   for tb in range(NT)] for j in range(5)]
            # V in [t, hs] layout, one tile per t-block: col u = t-chunk
            V = [glob.tile([128, 512], BF16, name=f"v{tb}", tag=f"v{tb}")
                 for tb in range(NT)]
            ONESB = glob.tile([128, 128], BF16)
            PERM = glob.tile([128, 128], BF16)
            IDN = glob.tile([128, 128], BF16)
            TRI = glob.tile([128, 128], BF16)
            COS = glob.tile([128, T], BF16)
            SIN = glob.tile([128, T], BF16)

            # ---------------- Phase A: qkv projection + rope -------------
            with tc.tile_pool(name="wa", bufs=1) as wap, \
                 tc.tile_pool(name="xp", bufs=4) as xp, \
                 tc.tile_pool(name="tmpa", bufs=2) as tmpa, \
                 tc.tile_pool(name="psA", bufs=1, space="PSUM") as psA, \
                 tc.tile_pool(name="psR", bufs=2, space="PSUM") as psR:
                # weight pair-chunks for DoubleRow: [p, hl, s, r] per n2
                WA = [wap.tile([128, 2 * 2 * RG], FP8, name=f"wa{n}",
                               tag=f"wa{n}")
                      for n in range(NC2)]
                # fp8 3-term: hi@hi + hi@lo + lo@hi (w_hl, x_hl)
                TERMS = ((0, 0), (0, 1), (1, 0))

                for tb in range(NT):
                    ts = slice(tb * 512, (tb + 1) * 512)
                    qkv_ps = [psA.tile([128, 512], F32, tag=f"qkv{j}",
                                       name=f"qkv{j}")
                              for j in range(NCOMP)]
                    for n in range(NC2):
                        if tb == 0:
                            for hl in range(2):
                                nc.sync.dma_start(
                                    WA[n][:].rearrange(
                                        "p (hl s r) -> p hl s r",
                                        hl=2, s=2)[:, hl],
                                    d_wa[hl, 256 * n:256 * (n + 1), :]
                                    .rearrange("(s p) r -> p s r", s=2))
                        xt = xp.tile([128, 2 * 2 * 512], FP8, tag="x")
                        for hl in range(2):
                            # x loads ride the idle Pool engine's SWDGE
                            # queue so HWDGE/SP stay free for weight loads
                            nc.gpsimd.dma_start(
                                xt[:].rearrange("p (hl s t) -> p hl s t",
                                                hl=2, s=2)[:, hl],
                                d_x[hl, 256 * n:256 * (n + 1), ts]
                                .rearrange("(s p) t -> p s t", s=2))
                        Wv = WA[n][:].rearrange("p (hl s r) -> p hl s r",
                                                hl=2, s=2)
                        Xv = xt[:].rearrange("p (hl s t) -> p hl s t",
                                             hl=2, s=2)
                        for j in range(NCOMP):
                            for ti, (wi, xi) in enumerate(TERMS):
                                nc.tensor.matmul(
                                    qkv_ps[j][:],
                                    Wv[:, wi, :, j * HS:(j + 1) * HS],
                                    Xv[:, xi],
                                    start=(n == 0 and ti == 0),
                                    stop=(n == NC2 - 1 and ti == 2),
                                    perf_mode=DR)

                    if tb == 0:
                        nc.sync.dma_start(ONESB[:], d_ones[:])
                        nc.sync.dma_start(PERM[:], d_perm[:])
                        nc.sync.dma_start(IDN[:], d_idn[:])
                        nc.sync.dma_start(TRI[:], d_tri[:])
                        nc.sync.dma_start(COS[:], d_cos[:])
                        nc.sync.dma_start(SIN[:], d_sin[:])

                    for j in range(5):  # q0..q3, k get rope
                        raw = tmpa.tile([128, 512], BF16, tag="raw")
                        nc.scalar.mul(raw[:], qkv_ps[j][:], 1.0 / WSCALE)
                        rot = psR.tile([128, 512], F32, tag="rot")
                        nc.tensor.matmul(rot[:], PERM[:], raw[:],
                                         start=True, stop=True)
                        t1 = tmpa.tile([128, 512], BF16, tag="t1")
                        nc.vector.tensor_tensor(t1[:], raw[:], COS[:, ts],
                                                mybir.AluOpType.mult)
                        t2 = tmpa.tile([128, 512], BF16, tag="t2")
                        nc.vector.tensor_tensor(t2[:], rot[:], SIN[:, ts],
                                                mybir.AluOpType.mult)
                        nc.vector.tensor_tensor(QQ[j][tb][:], t1[:], t2[:],
                                                mybir.AluOpType.add)

                    # v: transpose [hs, t] -> [t, hs] chunks
                    vraw = tmpa.tile([128, 512], BF16, tag="raw")
                    nc.scalar.mul(vraw[:], qkv_ps[5][:], 1.0 / WSCALE)
                    for u in range(4):
                        vt = psR.tile([128, 128], BF16, tag="rot")
                        nc.tensor.transpose(vt[:], vraw[:, u * 128:(u + 1) * 128],
                                            IDN[:])
                        nc.scalar.copy(V[tb][:, u * 128:(u + 1) * 128], vt[:])

            # ---------------- Phase B: causal attention ------------------
            with tc.tile_pool(name="wp", bufs=1) as wpp, \
                 tc.tile_pool(name="ptp", bufs=6) as ptp, \
                 tc.tile_pool(name="rcp", bufs=2) as rcp:
                WP = wpp.tile([128, 2 * QPK * C], FP8)   # [p, hl, h, c]
                for hl in range(2):
                    nc.sync.dma_start(
                        WP[:].rearrange("p (hl h c) -> p hl h c",
                                        hl=2, h=QPK)[:, hl],
                        d_wp[hl].rearrange("(h p) c -> p h c", h=QPK))
                Y8 = wpp.tile([128, 2 * QPK * T], FP8)   # [p, hl, h, t]

                with tc.tile_pool(name="psS", bufs=2, space="PSUM") as psS, \
                     tc.tile_pool(name="psY", bufs=2, space="PSUM") as psY, \
                     tc.tile_pool(name="psD", bufs=2, space="PSUM") as psD, \
                     tc.tile_pool(name="psO", bufs=2, space="PSUM") as psO, \
                     tc.tile_pool(name="outp", bufs=2) as outp:
                    Yv = Y8[:].rearrange(
                        "p (hl h t) -> p hl h t", hl=2, h=QPK)
                    WPv = WP[:].rearrange("p (hl h c) -> p hl h c",
                                          hl=2, h=QPK)
                    CTERMS = ((0, 0), (1, 0), (0, 1))   # (y_hl, w_hl)
                    for b in range(NT):
                        for h in range(QPK):
                            nkt = 4 * (b + 1)
                            y_ps = psY.tile([128, 512], F32, tag="y")
                            d_ps = psD.tile([128, 512], F32, tag="d")
                            for kt in range(nkt):
                                r = kt - 4 * b
                                # exact causal narrowing: tile r covers
                                # columns >= r*128; the leading 128-wide
                                # strip gets the triangular mask
                                off = 0 if r < 0 else r * 128
                                s_ps = psS.tile([128, 512], F32, tag="s")
                                nc.tensor.matmul(
                                    s_ps[:, off:],
                                    QQ[4][kt // 4][:, (kt % 4) * 128:
                                                   (kt % 4 + 1) * 128],
                                    QQ[h][b][:, off:],
                                    start=True, stop=True)
                                p_sb = ptp.tile([128, 512], BF16, tag="p")
                                nc.scalar.activation(
                                    p_sb[:, off:], s_ps[:, off:],
                                    mybir.ActivationFunctionType.Exp,
                                    scale=SCALE)
                                if r >= 0:
                                    nc.vector.tensor_tensor(
                                        p_sb[:, off:off + 128],
                                        p_sb[:, off:off + 128],
                                        TRI[:],
                                        mybir.AluOpType.mult)
                                nc.tensor.matmul(
                                    y_ps[:, off:],
                                    V[kt // 4][:, (kt % 4) * 128:
                                               (kt % 4 + 1) * 128],
                                    p_sb[:, off:],
                                    start=(kt == 0), stop=(kt == nkt - 1))
                                # column sums broadcast to all 128 rows:
                                # every output row of ONESB^T @ P is sum_k P
                                nc.tensor.matmul(
                                    d_ps[:, off:], ONESB[:], p_sb[:, off:],
                                    start=(kt == 0), stop=(kt == nkt - 1))
                            recip = rcp.tile([128, 512], F32R, tag="r")
                            with nc.allow_low_precision(
                                    reason="float32r is float32-width"):
                                nc.vector.reciprocal(recip[:], d_ps[:])
                            y_bf = rcp.tile([128, 512], BF16, tag="ybf")
                            nc.vector.tensor_tensor(
                                y_bf[:], y_ps[:], recip[:],
                                mybir.AluOpType.mult)
                            bs = slice(b * 512, (b + 1) * 512)
                            nc.scalar.copy(Yv[:, 0, h, bs], y_bf[:])
                            with nc.allow_low_precision(
                                    reason="fp8 residual split"):
                                nc.vector.tensor_tensor(
                                    Yv[:, 1, h, bs], y_bf[:],
                                    Yv[:, 0, h, bs],
                                    mybir.AluOpType.subtract)

                        # ---- output projection for this t-block ----
                        # runs as soon as all 4 heads of block b are done,
                        # filling PE while the next block is Act-bound
                        for tt in range(4 * b, 4 * (b + 1)):
                            o_sb = outp.tile([128, C], BF16, tag="o")
                            for cb in range(C // 512):
                                o_ps = psO.tile([128, 512], F32, tag="o")
                                for hp in range(QPK // 2):
                                    hsl = slice(2 * hp, 2 * hp + 2)
                                    tsl = slice(tt * 128, (tt + 1) * 128)
                                    csl = slice(cb * 512, (cb + 1) * 512)
                                    for ti, (yi, wi) in enumerate(CTERMS):
                                        nc.tensor.matmul(
                                            o_ps[:],
                                            Yv[:, yi, hsl, tsl],
                                            WPv[:, wi, hsl, csl],
                                            start=(hp == 0 and ti == 0),
                                            stop=(hp == 1 and ti == 2),
                                            perf_mode=DR)
                                # alternate Act/DVE for PSUM->SBUF copies
                                if cb % 2 == 0:
                                    nc.scalar.copy(
                                        o_sb[:, cb * 512:(cb + 1) * 512],
                                        o_ps[:])
                                else:
                                    nc.vector.tensor_copy(
                                        o_sb[:, cb * 512:(cb + 1) * 512],
                                        o_ps[:])
                            nc.sync.dma_start(
                                d_out[tt * 128:(tt + 1) * 128, :], o_sb[:])
    nc.compile()
    return nc


def _split8(a):
    """fp8 hi/lo split: a ~ unit sigma -> [2, ...] e4m3 stack."""
    hi = a.astype(NP8)
    lo = (a - hi.astype(np.float32)).astype(NP8)
    return np.ascontiguousarray(np.stack([hi, lo]))


def _host_inputs(x, cos, sin, w_attn, w_proj):
    """Build per-core input maps (host-side shard + transpose prep)."""
    f = np.float32
    x8 = _split8(np.ascontiguousarray(x.reshape(T, C).T))            # [2, C, T]
    cost = np.ascontiguousarray(cos.T).astype(NPBF16)                # [HS, T]
    sgn = np.ones((HS, 1), f)
    sgn[:HS // 2] = -1.0
    sint = np.ascontiguousarray(sin.T * sgn).astype(NPBF16)          # signed sin
    # rot(x)=P@x in [d,t] layout; matmul computes lhsT.T @ rhs -> lhsT = P.T
    P = np.zeros((HS, HS), f)
    for i in range(HS // 2):
        P[i, i + HS // 2] = 1.0
        P[i + HS // 2, i] = 1.0
    perm = np.ascontiguousarray(P.T).astype(NPBF16)
    idn = np.eye(128, dtype=f).astype(NPBF16)
    ones = np.ones((128, 128), f).astype(NPBF16)
    # triangular strip mask: keep iff col >= row
    iidx = np.arange(128)
    tri = (iidx[None, :] >= iidx[:, None]).astype(f).astype(NPBF16)

    maps = []
    for g in range(N_CORES):
        wa8 = _split8(WSCALE *
                      np.ascontiguousarray(w_attn[g * RG:(g + 1) * RG, :].T))
        wp8 = _split8(WSCALE *
                      np.ascontiguousarray(w_proj[:, g * OG:(g + 1) * OG].T))
        maps.append({
            "x8": x8, "wa8": wa8, "wp8": wp8, "cost": cost, "sint": sint,
            "tri": tri, "perm": perm, "idn": idn, "ones": ones,
        })
    return maps


_PROGRAM = None


def kernel(x, cos, sin, w_attn, w_proj):
    global _PROGRAM
    if _PROGRAM is None:
        _PROGRAM = _build_program()
    maps = _host_inputs(np.asarray(x), np.asarray(cos), np.asarray(sin),
                        np.asarray(w_attn), np.asarray(w_proj))
    res = bass_utils.run_bass_kernel_spmd(_PROGRAM, maps, list(range(N_CORES)))
    out = np.zeros((T, C), np.float32)
    for g in range(N_CORES):
        out += np.asarray(res.results[g]["out"], dtype=np.float32)
    return (out / WSCALE).reshape(1, T, C)


# revision 38
# speedup vs baseline: 1.0390x; 1.0390x over previous
"""Trainium2 Bass kernel for CausalSelfAttention (B=1, T=2048, C=4096,
32 heads / 8 query groups / head_size 128, full-dim RoPE, GQA).

Sharding: tensor-parallel over the 8 query groups. Core g owns w_attn rows
[g*768:(g+1)*768] (4 q heads + 1 k + 1 v) and w_proj columns
[g*512:(g+1)*512]; x is replicated. Each core returns a partial projection
output [2048, 4096] (bf16); the host sums the 8 partials (the all-reduce).

v1: bf16 datapath (same PE cost as float32r but half DMA/SBUF, faster DVE),
softmax denominator on the GpSimd/Pool engine instead of PE matmuls, exact
causal narrowing on diagonal tiles, batched DMAs, bf16 staged output.
"""

import os
import sys

for _p in ("/opt/trn_rl_repo", "/root/.axon_site/_ro/trn_rl_repo"):
    if os.path.isdir(_p) and _p not in sys.path:
        sys.path.insert(0, _p)

import numpy as np
import ml_dtypes

import concourse.bass as bass
import concourse.mybir as mybir
import concourse.tile as tile
from concourse import bacc, bass_utils

N_CORES = 8
T = 2048
C = 4096
HS = 128
N_HEAD = 32
G = 8                      # query groups == cores
QPK = 4                    # q heads per group
NCOMP = QPK + 2            # q0..q3, k, v
RG = NCOMP * HS            # 768 w_attn rows per group
OG = QPK * HS              # 512 proj-input cols per group
NT = T // 512              # 4 blocks of 512 along t
NC = C // 128              # 32 contraction chunks
NQ = C // 512              # 8 contraction quads
SCALE = 1.0 / np.sqrt(float(HS))

F32 = mybir.dt.float32
F32R = mybir.dt.float32r
BF16 = mybir.dt.bfloat16
FP8 = mybir.dt.float8e4
NPBF16 = ml_dtypes.bfloat16
NP8 = ml_dtypes.float8_e4m3
DR = mybir.MatmulPerfMode.DoubleRow
NC2 = C // 256             # 16 double-row pair chunks
WSCALE = 64.0              # weight pre-scale so fp8 operands are ~unit sigma


def _build_program():
    nc = bacc.Bacc(trn_type="TRN2", target_bir_lowering=False, debug=False,
                   num_devices=N_CORES)

    d_x = nc.dram_tensor("x8", [2, C, T], FP8, kind="ExternalInput").ap()
    d_wa = nc.dram_tensor("wa8", [2, C, RG], FP8, kind="ExternalInput").ap()
    d_wp = nc.dram_tensor("wp8", [2, OG, C], FP8, kind="ExternalInput").ap()
    d_cos = nc.dram_tensor("cost", [HS, T], BF16, kind="ExternalInput").ap()
    d_sin = nc.dram_tensor("sint", [HS, T], BF16, kind="ExternalInput").ap()
    d_tri = nc.dram_tensor("tri", [128, 128], BF16, kind="ExternalInput").ap()
    d_perm = nc.dram_tensor("perm", [128, 128], BF16, kind="ExternalInput").ap()
    d_idn = nc.dram_tensor("idn", [128, 128], BF16, kind="ExternalInput").ap()
    d_ones = nc.dram_tensor("ones", [128, 128], BF16, kind="ExternalInput").ap()
    d_out = nc.dram_tensor("out", [T, C], BF16, kind="ExternalOutput").ap()

    with tile.TileContext(nc) as tc:
        with tc.tile_pool(name="glob", bufs=1) as glob:
            # roped q0..q3 / k, one tile per (comp, t-block): [hs=128, 512]
            QQ = [[glob.tile([128, 512], BF16, name=f"qq{j}_{tb}",
                             tag=f"qq{j}_{tb}")
                   for tb in range(NT)] for j in range(5)]
            # V in [t, hs] layout, one tile per t-block: col u = t-chunk
            V = [glob.tile([128, 512], BF16, name=f"v{tb}", tag=f"v{tb}")
                 for tb in range(NT)]
            ONESB = glob.tile([128, 128], BF16)
            PERM = glob.tile([128, 128], BF16)
            IDN = glob.tile([128, 128], BF16)
            TRI = glob.tile([128, 128], BF16)
            COS = glob.tile([128, T], BF16)
            SIN = glob.tile([128, T], BF16)

            # ---------------- Phase A: qkv projection + rope -------------
            with tc.tile_pool(name="wa", bufs=1) as wap, \
                 tc.tile_pool(name="xp", bufs=6) as xp, \
                 tc.tile_pool(name="rawp", bufs=2) as rawp, \
                 tc.tile_pool(name="tmpa", bufs=2) as tmpa, \
                 tc.tile_pool(name="psA", bufs=1, space="PSUM") as psA, \
                 tc.tile_pool(name="psR", bufs=2, space="PSUM") as psR:
                # weight pair-chunks for DoubleRow: [p, hl, s, r] per n2
                WA = [wap.tile([128, 2 * 2 * RG], FP8, name=f"wa{n}",
                               tag=f"wa{n}")
                      for n in range(NC2)]
                # fp8 3-term: hi@hi + hi@lo + lo@hi (w_hl, x_hl)
                TERMS = ((0, 0), (0, 1), (1, 0))

                def rope_item(tb, j, raw):
                    """PE/DVE tail of rope for comp j of block tb."""
                    ts = slice(tb * 512, (tb + 1) * 512)
                    def emit():
                        rot = psR.tile([128, 512], F32, tag="rot")
                        nc.tensor.matmul(rot[:], PERM[:], raw[:],
                                         start=True, stop=True)
                        t1 = tmpa.tile([128, 512], BF16, tag="t1")
                        nc.vector.tensor_tensor(t1[:], raw[:], COS[:, ts],
                                                mybir.AluOpType.mult)
                        t2 = tmpa.tile([128, 512], BF16, tag="t2")
                        nc.vector.tensor_tensor(t2[:], rot[:], SIN[:, ts],
                                                mybir.AluOpType.mult)
                        nc.vector.tensor_tensor(QQ[j][tb][:], t1[:], t2[:],
                                                mybir.AluOpType.add)
                    return emit

                def v_item(tb, vraw, u):
                    def emit():
                        vt = psR.tile([128, 128], BF16, tag="rot")
                        nc.tensor.transpose(
                            vt[:], vraw[:, u * 128:(u + 1) * 128], IDN[:])
                        nc.scalar.copy(V[tb][:, u * 128:(u + 1) * 128],
                                       vt[:])
                    return emit

                deferred = []
                for tb in range(NT):
                    ts = slice(tb * 512, (tb + 1) * 512)
                    qkv_ps = [psA.tile([128, 512], F32, tag=f"qkv{j}",
                                       name=f"qkv{j}")
                              for j in range(NCOMP)]
                    for n in range(NC2):
                        if tb == 0:
                            for hl in range(2):
                                nc.sync.dma_start(
                                    WA[n][:].rearrange(
                                        "p (hl s r) -> p hl s r",
                                        hl=2, s=2)[:, hl],
                                    d_wa[hl, 256 * n:256 * (n + 1), :]
                                    .rearrange("(s p) r -> p s r", s=2))
                        xt = xp.tile([128, 2 * 2 * 512], FP8, tag="x")
                        for hl in range(2):
                            # split DMA issue: hi plane on the idle Pool
                            # SWDGE queue, lo plane on SP/HWDGE, so neither
                            # generator paces the matmul loop
                            eng = nc.gpsimd if hl == 0 else nc.sync
                            eng.dma_start(
                                xt[:].rearrange("p (hl s t) -> p hl s t",
                                                hl=2, s=2)[:, hl],
                                d_x[hl, 256 * n:256 * (n + 1), ts]
                                .rearrange("(s p) t -> p s t", s=2))
                        Wv = WA[n][:].rearrange("p (hl s r) -> p hl s r",
                                                hl=2, s=2)
                        Xv = xt[:].rearrange("p (hl s t) -> p hl s t",
                                             hl=2, s=2)
                        for j in range(NCOMP):
                            for ti, (wi, xi) in enumerate(TERMS):
                                nc.tensor.matmul(
                                    qkv_ps[j][:],
                                    Wv[:, wi, :, j * HS:(j + 1) * HS],
                                    Xv[:, xi],
                                    start=(n == 0 and ti == 0),
                                    stop=(n == NC2 - 1 and ti == 2),
                                    perf_mode=DR)
                        # spread the previous block's rope/transpose PE
                        # work between this block's GEMM chunks so the PE
                        # queue never drains on the Act/DVE rope chain
                        if deferred and n < len(deferred):
                            deferred[n]()

                    deferred = []
                    if tb == 0:
                        nc.sync.dma_start(ONESB[:], d_ones[:])
                        nc.sync.dma_start(PERM[:], d_perm[:])
                        nc.sync.dma_start(IDN[:], d_idn[:])
                        nc.sync.dma_start(TRI[:], d_tri[:])
                        nc.sync.dma_start(COS[:], d_cos[:])
                        nc.sync.dma_start(SIN[:], d_sin[:])

                    for j in range(5):  # q0..q3, k get rope
                        raw = rawp.tile([128, 512], BF16, tag=f"raw{j}")
                        nc.scalar.mul(raw[:], qkv_ps[j][:], 1.0 / WSCALE)
                        deferred.append(rope_item(tb, j, raw))

                    # v: transpose [hs, t] -> [t, hs] chunks
                    vraw = rawp.tile([128, 512], BF16, tag="raw5")
                    nc.scalar.mul(vraw[:], qkv_ps[5][:], 1.0 / WSCALE)
                    for u in range(4):
                        deferred.append(v_item(tb, vraw, u))
                for it in deferred:   # flush the last block's rope
                    it()

            # ---------------- Phase B: causal attention ------------------
            with tc.tile_pool(name="wp", bufs=1) as wpp, \
                 tc.tile_pool(name="ptp", bufs=6) as ptp, \
                 tc.tile_pool(name="rcp", bufs=2) as rcp:
                WP = wpp.tile([128, 2 * QPK * C], FP8)   # [p, hl, h, c]
                for hl in range(2):
                    nc.sync.dma_start(
                        WP[:].rearrange("p (hl h c) -> p hl h c",
                                        hl=2, h=QPK)[:, hl],
                        d_wp[hl].rearrange("(h p) c -> p h c", h=QPK))
                Y8 = wpp.tile([128, 2 * QPK * T], FP8)   # [p, hl, h, t]

                with tc.tile_pool(name="psS", bufs=2, space="PSUM") as psS, \
                     tc.tile_pool(name="psY", bufs=2, space="PSUM") as psY, \
                     tc.tile_pool(name="psD", bufs=2, space="PSUM") as psD, \
                     tc.tile_pool(name="psO", bufs=2, space="PSUM") as psO, \
                     tc.tile_pool(name="outp", bufs=2) as outp:
                    Yv = Y8[:].rearrange(
                        "p (hl h t) -> p hl h t", hl=2, h=QPK)
                    WPv = WP[:].rearrange("p (hl h c) -> p hl h c",
                                          hl=2, h=QPK)
                    CTERMS = ((0, 0), (1, 0), (0, 1))   # (y_hl, w_hl)
                    for b in range(NT):
                        for h in range(QPK):
                            nkt = 4 * (b + 1)
                            y_ps = psY.tile([128, 512], F32, tag="y")
                            d_ps = psD.tile([128, 512], F32, tag="d")
                            # software pipeline: issue QK(kt) before
                            # AV/D(kt-1) so PE rolls past the exp latency
                            pend = None
                            for kt in range(nkt):
                                r = kt - 4 * b
                                # exact causal narrowing: tile r covers
                                # columns >= r*128; the leading 128-wide
                                # strip gets the triangular mask
                                off = 0 if r < 0 else r * 128
                                s_ps = psS.tile([128, 512], F32, tag="s")
                                nc.tensor.matmul(
                                    s_ps[:, off:],
                                    QQ[4][kt // 4][:, (kt % 4) * 128:
                                                   (kt % 4 + 1) * 128],
                                    QQ[h][b][:, off:],
                                    start=True, stop=True)
                                p_sb = ptp.tile([128, 512], BF16, tag="p")
                                if pend is not None:
                                    kp, offp, pp = pend
                                    nc.tensor.matmul(
                                        y_ps[:, offp:],
                                        V[kp // 4][:, (kp % 4) * 128:
                                                   (kp % 4 + 1) * 128],
                                        pp[:, offp:],
                                        start=(kp == 0), stop=False)
                                    nc.tensor.matmul(
                                        d_ps[:, offp:], ONESB[:],
                                        pp[:, offp:],
                                        start=(kp == 0), stop=False)
                                nc.scalar.activation(
                                    p_sb[:, off:], s_ps[:, off:],
                                    mybir.ActivationFunctionType.Exp,
                                    scale=SCALE)
                                if r >= 0:
                                    nc.vector.tensor_tensor(
                                        p_sb[:, off:off + 128],
                                        p_sb[:, off:off + 128],
                                        TRI[:],
                                        mybir.AluOpType.mult)
                                pend = (kt, off, p_sb)
                            kp, offp, pp = pend
                            nc.tensor.matmul(
                                y_ps[:, offp:],
                                V[kp // 4][:, (kp % 4) * 128:
                                           (kp % 4 + 1) * 128],
                                pp[:, offp:],
                                start=(kp == 0), stop=True)
                            nc.tensor.matmul(
                                d_ps[:, offp:], ONESB[:], pp[:, offp:],
                                start=(kp == 0), stop=True)
                            recip = rcp.tile([128, 512], F32R, tag="r")
                            with nc.allow_low_precision(
                                    reason="float32r is float32-width"):
                                nc.vector.reciprocal(recip[:], d_ps[:])
                            y_bf = rcp.tile([128, 512], BF16, tag="ybf")
                            nc.vector.tensor_tensor(
                                y_bf[:], y_ps[:], recip[:],
                                mybir.AluOpType.mult)
                            bs = slice(b * 512, (b + 1) * 512)
                            nc.scalar.copy(Yv[:, 0, h, bs], y_bf[:])
                            with nc.allow_low_precision(
                                    reason="fp8 residual split"):
                                nc.vector.tensor_tensor(
                                    Yv[:, 1, h, bs], y_bf[:],
                                    Yv[:, 0, h, bs],
                                    mybir.AluOpType.subtract)

                        # ---- output projection for this t-block ----
                        # runs as soon as all 4 heads of block b are done,
                        # filling PE while the next block is Act-bound
                        for tt in range(4 * b, 4 * (b + 1)):
                            o_sb = outp.tile([128, C], BF16, tag="o")
                            for cb in range(C // 512):
                                o_ps = psO.tile([128, 512], F32, tag="o")
                                for hp in range(QPK // 2):
                                    hsl = slice(2 * hp, 2 * hp + 2)
                                    tsl = slice(tt * 128, (tt + 1) * 128)
                                    csl = slice(cb * 512, (cb + 1) * 512)
                                    for ti, (yi, wi) in enumerate(CTERMS):
                                        nc.tensor.matmul(
                                            o_ps[:],
                                            Yv[:, yi, hsl, tsl],
                                            WPv[:, wi, hsl, csl],
                                            start=(hp == 0 and ti == 0),
                                            stop=(hp == 1 and ti == 2),
                                            perf_mode=DR)
                                # alternate Act/DVE for PSUM->SBUF copies
                                if cb % 2 == 0:
                                    nc.scalar.copy(
                                        o_sb[:, cb * 512:(cb + 1) * 512],
                                        o_ps[:])
                                else:
                                    nc.vector.tensor_copy(
                                        o_sb[:, cb * 512:(cb + 1) * 512],
                                        o_ps[:])
                                if cb % 2 == 1:
                                    cs0 = (cb - 1) * 512
                                    nc.sync.dma_start(
                                        d_out[tt * 128:(tt + 1) * 128,
                                              cs0:cs0 + 1024],
                                        o_sb[:, cs0:cs0 + 1024])
    nc.compile()
    return nc


def _split8(a):
    """fp8 hi/lo split: a ~ unit sigma -> [2, ...] e4m3 stack."""
    hi = a.astype(NP8)
    lo = (a - hi.astype(np.float32)).astype(NP8)
    return np.ascontiguousarray(np.stack([hi, lo]))


def _host_inputs(x, cos, sin, w_attn, w_proj):
    """Build per-core input maps (host-side shard + transpose prep)."""
    f = np.float32
    x8 = _split8(np.ascontiguousarray(x.reshape(T, C).T))            # [2, C, T]
    cost = np.ascontiguousarray(cos.T).astype(NPBF16)                # [HS, T]
    sgn = np.ones((HS, 1), f)
    sgn[:HS // 2] = -1.0
    sint = np.ascontiguousarray(sin.T * sgn).astype(NPBF16)          # signed sin
    # rot(x)=P@x in [d,t] layout; matmul computes lhsT.T @ rhs -> lhsT = P.T
    P = np.zeros((HS, HS), f)
    for i in range(HS // 2):
        P[i, i + HS // 2] = 1.0
        P[i + HS // 2, i] = 1.0
    perm = np.ascontiguousarray(P.T).astype(NPBF16)
    idn = np.eye(128, dtype=f).astype(NPBF16)
    ones = np.ones((128, 128), f).astype(NPBF16)
    # triangular strip mask: keep iff col >= row
    iidx = np.arange(128)
    tri = (iidx[None, :] >= iidx[:, None]).astype(f).astype(NPBF16)

    maps = []
    for g in range(N_CORES):
        wa8 = _split8(WSCALE *
                      np.ascontiguousarray(w_attn[g * RG:(g + 1) * RG, :].T))
        wp8 = _split8(WSCALE *
                      np.ascontiguousarray(w_proj[:, g * OG:(g + 1) * OG].T))
        maps.append({
            "x8": x8, "wa8": wa8, "wp8": wp8, "cost": cost, "sint": sint,
            "tri": tri, "perm": perm, "idn": idn, "ones": ones,
        })
    return maps


_PROGRAM = None


def kernel(x, cos, sin, w_attn, w_proj):
    global _PROGRAM
    if _PROGRAM is None:
        _PROGRAM = _build_program()
    maps = _host_inputs(np.asarray(x), np.asarray(cos), np.asarray(sin),
                        np.asarray(w_attn), np.asarray(w_proj))
    res = bass_utils.run_bass_kernel_spmd(_PROGRAM, maps, list(range(N_CORES)))
    out = np.zeros((T, C), np.float32)
    for g in range(N_CORES):
        out += np.asarray(res.results[g]["out"], dtype=np.float32)
    return (out / WSCALE).reshape(1, T, C)


# revision 40
# speedup vs baseline: 1.0513x; 1.0119x over previous
"""Trainium2 Bass kernel for CausalSelfAttention (B=1, T=2048, C=4096,
32 heads / 8 query groups / head_size 128, full-dim RoPE, GQA).

Sharding: tensor-parallel over the 8 query groups. Core g owns w_attn rows
[g*768:(g+1)*768] (4 q heads + 1 k + 1 v) and w_proj columns
[g*512:(g+1)*512]; x is replicated. Each core returns a partial projection
output [2048, 4096] (bf16); the host sums the 8 partials (the all-reduce).

v1: bf16 datapath (same PE cost as float32r but half DMA/SBUF, faster DVE),
softmax denominator on the GpSimd/Pool engine instead of PE matmuls, exact
causal narrowing on diagonal tiles, batched DMAs, bf16 staged output.
"""

import os
import sys

for _p in ("/opt/trn_rl_repo", "/root/.axon_site/_ro/trn_rl_repo"):
    if os.path.isdir(_p) and _p not in sys.path:
        sys.path.insert(0, _p)

import numpy as np
import ml_dtypes

import concourse.bass as bass
import concourse.mybir as mybir
import concourse.tile as tile
from concourse import bacc, bass_utils

N_CORES = 8
T = 2048
C = 4096
HS = 128
N_HEAD = 32
G = 8                      # query groups == cores
QPK = 4                    # q heads per group
NCOMP = QPK + 2            # q0..q3, k, v
RG = NCOMP * HS            # 768 w_attn rows per group
OG = QPK * HS              # 512 proj-input cols per group
NT = T // 512              # 4 blocks of 512 along t
NC = C // 128              # 32 contraction chunks
NQ = C // 512              # 8 contraction quads
SCALE = 1.0 / np.sqrt(float(HS))

F32 = mybir.dt.float32
F32R = mybir.dt.float32r
BF16 = mybir.dt.bfloat16
FP8 = mybir.dt.float8e4
NPBF16 = ml_dtypes.bfloat16
NP8 = ml_dtypes.float8_e4m3
DR = mybir.MatmulPerfMode.DoubleRow
NC2 = C // 256             # 16 double-row pair chunks
WSCALE = 64.0              # weight pre-scale so fp8 operands are ~unit sigma


def _build_program():
    nc = bacc.Bacc(trn_type="TRN2", target_bir_lowering=False, debug=False,
                   num_devices=N_CORES)

    d_x = nc.dram_tensor("x8", [2, C, T], FP8, kind="ExternalInput").ap()
    d_wa = nc.dram_tensor("wa8", [2, C, RG], FP8, kind="ExternalInput").ap()
    d_wp = nc.dram_tensor("wp8", [2, OG, C], FP8, kind="ExternalInput").ap()
    d_cos = nc.dram_tensor("cost", [HS, T], BF16, kind="ExternalInput").ap()
    d_sin = nc.dram_tensor("sint", [HS, T], BF16, kind="ExternalInput").ap()
    d_tri = nc.dram_tensor("tri", [128, 128], BF16, kind="ExternalInput").ap()
    d_perm = nc.dram_tensor("perm", [128, 128], BF16, kind="ExternalInput").ap()
    d_idn = nc.dram_tensor("idn", [128, 128], BF16, kind="ExternalInput").ap()
    d_ones = nc.dram_tensor("ones", [128, 128], BF16, kind="ExternalInput").ap()
    d_out = nc.dram_tensor("out", [T, C], BF16, kind="ExternalOutput").ap()

    with tile.TileContext(nc) as tc:
        with tc.tile_pool(name="glob", bufs=1) as glob:
            # roped q0..q3 / k, one tile per (comp, t-block): [hs=128, 512]
            QQ = [[glob.tile([128, 512], BF16, name=f"qq{j}_{tb}",
                             tag=f"qq{j}_{tb}")
                   for tb in range(NT)] for j in range(5)]
            # V in [t, hs] layout, one tile per t-block: col u = t-chunk
            V = [glob.tile([128, 512], BF16, name=f"v{tb}", tag=f"v{tb}")
                 for tb in range(NT)]
            ONESB = glob.tile([128, 128], BF16)
            PERM = glob.tile([128, 128], BF16)
            IDN = glob.tile([128, 128], BF16)
            TRI = glob.tile([128, 128], BF16)
            COS = glob.tile([128, T], BF16)
            SIN = glob.tile([128, T], BF16)

            # ---------------- Phase A: qkv projection + rope -------------
            with tc.tile_pool(name="wa", bufs=1) as wap, \
                 tc.tile_pool(name="xp", bufs=10) as xp, \
                 tc.tile_pool(name="rawp", bufs=2) as rawp, \
                 tc.tile_pool(name="tmpa", bufs=2) as tmpa, \
                 tc.tile_pool(name="psA", bufs=1, space="PSUM") as psA, \
                 tc.tile_pool(name="psR", bufs=2, space="PSUM") as psR:
                # weight pair-chunks for DoubleRow: [p, hl, s, r] per n2
                WA = [wap.tile([128, 2 * 2 * RG], FP8, name=f"wa{n}",
                               tag=f"wa{n}")
                      for n in range(NC2)]
                # fp8 3-term: hi@hi + hi@lo + lo@hi (w_hl, x_hl)
                TERMS = ((0, 0), (0, 1), (1, 0))

                def rope_item(tb, j, raw):
                    """PE/DVE tail of rope for comp j of block tb."""
                    ts = slice(tb * 512, (tb + 1) * 512)
                    def emit():
                        rot = psR.tile([128, 512], F32, tag="rot")
                        nc.tensor.matmul(rot[:], PERM[:], raw[:],
                                         start=True, stop=True)
                        t1 = tmpa.tile([128, 512], BF16, tag="t1")
                        nc.vector.tensor_tensor(t1[:], raw[:], COS[:, ts],
                                                mybir.AluOpType.mult)
                        t2 = tmpa.tile([128, 512], BF16, tag="t2")
                        nc.vector.tensor_tensor(t2[:], rot[:], SIN[:, ts],
                                                mybir.AluOpType.mult)
                        nc.vector.tensor_tensor(QQ[j][tb][:], t1[:], t2[:],
                                                mybir.AluOpType.add)
                    return emit

                def v_item(tb, vraw, u):
                    def emit():
                        vt = psR.tile([128, 128], BF16, tag="rot")
                        nc.tensor.transpose(
                            vt[:], vraw[:, u * 128:(u + 1) * 128], IDN[:])
                        nc.scalar.copy(V[tb][:, u * 128:(u + 1) * 128],
                                       vt[:])
                    return emit

                deferred = []
                for tb in range(NT):
                    ts = slice(tb * 512, (tb + 1) * 512)
                    qkv_ps = [psA.tile([128, 512], F32, tag=f"qkv{j}",
                                       name=f"qkv{j}")
                              for j in range(NCOMP)]
                    for n in range(NC2):
                        if tb == 0:
                            for hl in range(2):
                                nc.sync.dma_start(
                                    WA[n][:].rearrange(
                                        "p (hl s r) -> p hl s r",
                                        hl=2, s=2)[:, hl],
                                    d_wa[hl, 256 * n:256 * (n + 1), :]
                                    .rearrange("(s p) r -> p s r", s=2))
                        xt = xp.tile([128, 2 * 2 * 512], FP8, tag="x")
                        for hl in range(2):
                            # split DMA issue: hi plane on the idle Pool
                            # SWDGE queue, lo plane on SP/HWDGE, so neither
                            # generator paces the matmul loop
                            eng = nc.gpsimd if hl == 0 else nc.sync
                            eng.dma_start(
                                xt[:].rearrange("p (hl s t) -> p hl s t",
                                                hl=2, s=2)[:, hl],
                                d_x[hl, 256 * n:256 * (n + 1), ts]
                                .rearrange("(s p) t -> p s t", s=2))
                        Wv = WA[n][:].rearrange("p (hl s r) -> p hl s r",
                                                hl=2, s=2)
                        Xv = xt[:].rearrange("p (hl s t) -> p hl s t",
                                             hl=2, s=2)
                        for j in range(NCOMP):
                            for ti, (wi, xi) in enumerate(TERMS):
                                nc.tensor.matmul(
                                    qkv_ps[j][:],
                                    Wv[:, wi, :, j * HS:(j + 1) * HS],
                                    Xv[:, xi],
                                    start=(n == 0 and ti == 0),
                                    stop=(n == NC2 - 1 and ti == 2),
                                    perf_mode=DR)
                        # spread the previous block's rope/transpose PE
                        # work between this block's GEMM chunks so the PE
                        # queue never drains on the Act/DVE rope chain
                        if deferred and n < len(deferred):
                            deferred[n]()
                        # spread the constant loads through tb0's compute
                        # so they don't contend with x loads at the tb0/tb1
                        # boundary
                        if tb == 0 and 4 <= n < 10:
                            cdma = [(PERM, d_perm), (IDN, d_idn),
                                    (ONESB, d_ones), (TRI, d_tri),
                                    (COS, d_cos), (SIN, d_sin)][n - 4]
                            eng = nc.gpsimd if n % 2 == 0 else nc.sync
                            eng.dma_start(cdma[0][:], cdma[1][:])

                    deferred = []

                    for j in range(5):  # q0..q3, k get rope
                        raw = rawp.tile([128, 512], BF16, tag=f"raw{j}")
                        nc.scalar.mul(raw[:], qkv_ps[j][:], 1.0 / WSCALE)
                        deferred.append(rope_item(tb, j, raw))

                    # v: transpose [hs, t] -> [t, hs] chunks
                    vraw = rawp.tile([128, 512], BF16, tag="raw5")
                    nc.scalar.mul(vraw[:], qkv_ps[5][:], 1.0 / WSCALE)
                    for u in range(4):
                        deferred.append(v_item(tb, vraw, u))
                for it in deferred:   # flush the last block's rope
                    it()

            # ---------------- Phase B: causal attention ------------------
            with tc.tile_pool(name="wp", bufs=1) as wpp, \
                 tc.tile_pool(name="ptp", bufs=6) as ptp, \
                 tc.tile_pool(name="rcp", bufs=2) as rcp:
                WP = wpp.tile([128, 2 * QPK * C], FP8)   # [p, hl, h, c]
                for hl in range(2):
                    nc.sync.dma_start(
                        WP[:].rearrange("p (hl h c) -> p hl h c",
                                        hl=2, h=QPK)[:, hl],
                        d_wp[hl].rearrange("(h p) c -> p h c", h=QPK))
                Y8 = wpp.tile([128, 2 * QPK * T], FP8)   # [p, hl, h, t]

                with tc.tile_pool(name="psS", bufs=2, space="PSUM") as psS, \
                     tc.tile_pool(name="psY", bufs=2, space="PSUM") as psY, \
                     tc.tile_pool(name="psD", bufs=2, space="PSUM") as psD, \
                     tc.tile_pool(name="psO", bufs=2, space="PSUM") as psO, \
                     tc.tile_pool(name="outp", bufs=2) as outp:
                    Yv = Y8[:].rearrange(
                        "p (hl h t) -> p hl h t", hl=2, h=QPK)
                    WPv = WP[:].rearrange("p (hl h c) -> p hl h c",
                                          hl=2, h=QPK)
                    CTERMS = ((0, 0), (1, 0), (0, 1))   # (y_hl, w_hl)
                    for b in range(NT):
                        for h in range(QPK):
                            nkt = 4 * (b + 1)
                            y_ps = psY.tile([128, 512], F32, tag="y")
                            d_ps = psD.tile([128, 512], F32, tag="d")
                            # software pipeline: issue QK(kt) before
                            # AV/D(kt-1) so PE rolls past the exp latency
                            pend = None
                            for kt in range(nkt):
                                r = kt - 4 * b
                                # exact causal narrowing: tile r covers
                                # columns >= r*128; the leading 128-wide
                                # strip gets the triangular mask
                                off = 0 if r < 0 else r * 128
                                s_ps = psS.tile([128, 512], F32, tag="s")
                                nc.tensor.matmul(
                                    s_ps[:, off:],
                                    QQ[4][kt // 4][:, (kt % 4) * 128:
                                                   (kt % 4 + 1) * 128],
                                    QQ[h][b][:, off:],
                                    start=True, stop=True)
                                p_sb = ptp.tile([128, 512], BF16, tag="p")
                                if pend is not None:
                                    kp, offp, pp = pend
                                    nc.tensor.matmul(
                                        y_ps[:, offp:],
                                        V[kp // 4][:, (kp % 4) * 128:
                                                   (kp % 4 + 1) * 128],
                                        pp[:, offp:],
                                        start=(kp == 0), stop=False)
                                    nc.tensor.matmul(
                                        d_ps[:, offp:], ONESB[:],
                                        pp[:, offp:],
                                        start=(kp == 0), stop=False)
                                nc.scalar.activation(
                                    p_sb[:, off:], s_ps[:, off:],
                                    mybir.ActivationFunctionType.Exp,
                                    scale=SCALE)
                                if r >= 0:
                                    nc.vector.tensor_tensor(
                                        p_sb[:, off:off + 128],
                                        p_sb[:, off:off + 128],
                                        TRI[:],
                                        mybir.AluOpType.mult)
                                pend = (kt, off, p_sb)
                            kp, offp, pp = pend
                            nc.tensor.matmul(
                                y_ps[:, offp:],
                                V[kp // 4][:, (kp % 4) * 128:
                                           (kp % 4 + 1) * 128],
                                pp[:, offp:],
                                start=(kp == 0), stop=True)
                            nc.tensor.matmul(
                                d_ps[:, offp:], ONESB[:], pp[:, offp:],
                                start=(kp == 0), stop=True)
                            recip = rcp.tile([128, 512], F32R, tag="r")
                            with nc.allow_low_precision(
                                    reason="float32r is float32-width"):
                                nc.vector.reciprocal(recip[:], d_ps[:])
                            y_bf = rcp.tile([128, 512], BF16, tag="ybf")
                            nc.vector.tensor_tensor(
                                y_bf[:], y_ps[:], recip[:],
                                mybir.AluOpType.mult)
                            bs = slice(b * 512, (b + 1) * 512)
                            nc.scalar.copy(Yv[:, 0, h, bs], y_bf[:])
                            with nc.allow_low_precision(
                                    reason="fp8 residual split"):
                                nc.vector.tensor_tensor(
                                    Yv[:, 1, h, bs], y_bf[:],
                                    Yv[:, 0, h, bs],
                                    mybir.AluOpType.subtract)

                        # ---- output projection for this t-block ----
                        # runs as soon as all 4 heads of block b are done,
                        # filling PE while the next block is Act-bound
                        for tt in range(4 * b, 4 * (b + 1)):
                            o_sb = outp.tile([128, C], BF16, tag="o")
                            for cb in range(C // 512):
                                o_ps = psO.tile([128, 512], F32, tag="o")
                                for hp in range(QPK // 2):
                                    hsl = slice(2 * hp, 2 * hp + 2)
                                    tsl = slice(tt * 128, (tt + 1) * 128)
                                    csl = slice(cb * 512, (cb + 1) * 512)
                                    for ti, (yi, wi) in enumerate(CTERMS):
                                        nc.tensor.matmul(
                                            o_ps[:],
                                            Yv[:, yi, hsl, tsl],
                                            WPv[:, wi, hsl, csl],
                                            start=(hp == 0 and ti == 0),
                                            stop=(hp == 1 and ti == 2),
                                            perf_mode=DR)
                                # alternate Act/DVE for PSUM->SBUF copies
                                if cb % 2 == 0:
                                    nc.scalar.copy(
                                        o_sb[:, cb * 512:(cb + 1) * 512],
                                        o_ps[:])
                                else:
                                    nc.vector.tensor_copy(
                                        o_sb[:, cb * 512:(cb + 1) * 512],
                                        o_ps[:])
                                if cb % 2 == 1:
                                    cs0 = (cb - 1) * 512
                                    nc.sync.dma_start(
                                        d_out[tt * 128:(tt + 1) * 128,
                                              cs0:cs0 + 1024],
                                        o_sb[:, cs0:cs0 + 1024])
    nc.compile()
    return nc


def _split8(a):
    """fp8 hi/lo split: a ~ unit sigma -> [2, ...] e4m3 stack."""
    hi = a.astype(NP8)
    lo = (a - hi.astype(np.float32)).astype(NP8)
    return np.ascontiguousarray(np.stack([hi, lo]))


def _host_inputs(x, cos, sin, w_attn, w_proj):
    """Build per-core input maps (host-side shard + transpose prep)."""
    f = np.float32
    x8 = _split8(np.ascontiguousarray(x.reshape(T, C).T))            # [2, C, T]
    cost = np.ascontiguousarray(cos.T).astype(NPBF16)                # [HS, T]
    sgn = np.ones((HS, 1), f)
    sgn[:HS // 2] = -1.0
    sint = np.ascontiguousarray(sin.T * sgn).astype(NPBF16)          # signed sin
    # rot(x)=P@x in [d,t] layout; matmul computes lhsT.T @ rhs -> lhsT = P.T
    P = np.zeros((HS, HS), f)
    for i in range(HS // 2):
        P[i, i + HS // 2] = 1.0
        P[i + HS // 2, i] = 1.0
    perm = np.ascontiguousarray(P.T).astype(NPBF16)
    idn = np.eye(128, dtype=f).astype(NPBF16)
    ones = np.ones((128, 128), f).astype(NPBF16)
    # triangular strip mask: keep iff col >= row
    iidx = np.arange(128)
    tri = (iidx[None, :] >= iidx[:, None]).astype(f).astype(NPBF16)

    maps = []
    for g in range(N_CORES):
        wa8 = _split8(WSCALE *
                      np.ascontiguousarray(w_attn[g * RG:(g + 1) * RG, :].T))
        wp8 = _split8(WSCALE *
                      np.ascontiguousarray(w_proj[:, g * OG:(g + 1) * OG].T))
        maps.append({
            "x8": x8, "wa8": wa8, "wp8": wp8, "cost": cost, "sint": sint,
            "tri": tri, "perm": perm, "idn": idn, "ones": ones,
        })
    return maps


_PROGRAM = None


def kernel(x, cos, sin, w_attn, w_proj):
    global _PROGRAM
    if _PROGRAM is None:
        _PROGRAM = _build_program()
    maps = _host_inputs(np.asarray(x), np.asarray(cos), np.asarray(sin),
                        np.asarray(w_attn), np.asarray(w_proj))
    res = bass_utils.run_bass_kernel_spmd(_PROGRAM, maps, list(range(N_CORES)))
    out = np.zeros((T, C), np.float32)
    for g in range(N_CORES):
        out += np.asarray(res.results[g]["out"], dtype=np.float32)
    return (out / WSCALE).reshape(1, T, C)


# revision 54
# speedup vs baseline: 1.0708x; 1.0185x over previous
"""Trainium2 Bass kernel for CausalSelfAttention (B=1, T=2048, C=4096,
32 heads / 8 query groups / head_size 128, full-dim RoPE, GQA).

Sharding: tensor-parallel over the 8 query groups. Core g owns w_attn rows
[g*768:(g+1)*768] (4 q heads + 1 k + 1 v) and w_proj columns
[g*512:(g+1)*512]; x is replicated. Each core returns a partial projection
output [2048, 4096] (bf16); the host sums the 8 partials (the all-reduce).

v1: bf16 datapath (same PE cost as float32r but half DMA/SBUF, faster DVE),
softmax denominator on the GpSimd/Pool engine instead of PE matmuls, exact
causal narrowing on diagonal tiles, batched DMAs, bf16 staged output.
"""

import os
import sys

for _p in ("/opt/trn_rl_repo", "/root/.axon_site/_ro/trn_rl_repo"):
    if os.path.isdir(_p) and _p not in sys.path:
        sys.path.insert(0, _p)

import numpy as np
import ml_dtypes

import concourse.bass as bass
import concourse.mybir as mybir
import concourse.tile as tile
from concourse import bacc, bass_utils

N_CORES = 8
T = 2048
C = 4096
HS = 128
N_HEAD = 32
G = 8                      # query groups == cores
QPK = 4                    # q heads per group
NCOMP = QPK + 2            # q0..q3, k, v
RG = NCOMP * HS            # 768 w_attn rows per group
OG = QPK * HS              # 512 proj-input cols per group
NT = T // 512              # 4 blocks of 512 along t
NC = C // 128              # 32 contraction chunks
NQ = C // 512              # 8 contraction quads
SCALE = 1.0 / np.sqrt(float(HS))

F32 = mybir.dt.float32
F32R = mybir.dt.float32r
BF16 = mybir.dt.bfloat16
FP8 = mybir.dt.float8e4
NPBF16 = ml_dtypes.bfloat16
NP8 = ml_dtypes.float8_e4m3
DR = mybir.MatmulPerfMode.DoubleRow
NC2 = C // 256             # 16 double-row pair chunks
WSCALE = 64.0              # weight pre-scale so fp8 operands are ~unit sigma


def _build_program():
    nc = bacc.Bacc(trn_type="TRN2", target_bir_lowering=False, debug=False,
                   num_devices=N_CORES)

    d_x = nc.dram_tensor("x8", [2, C, T], FP8, kind="ExternalInput").ap()
    d_wa = nc.dram_tensor("wa8", [2, C, RG], FP8, kind="ExternalInput").ap()
    d_wp = nc.dram_tensor("wp8", [2, OG, C], FP8, kind="ExternalInput").ap()
    d_cos = nc.dram_tensor("cost", [HS, T], BF16, kind="ExternalInput").ap()
    d_sin = nc.dram_tensor("sint", [HS, T], BF16, kind="ExternalInput").ap()
    d_tri = nc.dram_tensor("tri", [128, 128], BF16, kind="ExternalInput").ap()
    d_perm = nc.dram_tensor("perm", [128, 128], BF16, kind="ExternalInput").ap()
    d_idn = nc.dram_tensor("idn", [128, 128], BF16, kind="ExternalInput").ap()
    d_ones = nc.dram_tensor("ones", [128, 128], BF16, kind="ExternalInput").ap()
    d_ones32 = nc.dram_tensor("ones32", [128, 128], F32R,
                              kind="ExternalInput").ap()
    d_out = nc.dram_tensor("out", [T, C], BF16, kind="ExternalOutput").ap()

    with tile.TileContext(nc) as tc:
        with tc.tile_pool(name="glob", bufs=1) as glob:
            # roped q0..q3 / k, one tile per (comp, t-block): [hs=128, 512]
            QQ = [[glob.tile([128, 512], BF16, name=f"qq{j}_{tb}",
                             tag=f"qq{j}_{tb}")
                   for tb in range(NT)] for j in range(5)]
            # V in [t, hs] layout, one tile per t-block: col u = t-chunk
            V = [glob.tile([128, 512], BF16, name=f"v{tb}", tag=f"v{tb}")
                 for tb in range(NT)]
            ONESB = glob.tile([128, 128], BF16)
            ONES32 = glob.tile([128, 128], F32R)
            PERM = glob.tile([128, 128], BF16)
            IDN = glob.tile([128, 128], BF16)
            TRI = glob.tile([128, 128], BF16)
            COS = glob.tile([128, T], BF16)
            SIN = glob.tile([128, T], BF16)

            # ---------------- Phase A: qkv projection + rope -------------
            with tc.tile_pool(name="wa", bufs=1) as wap, \
                 tc.tile_pool(name="xp", bufs=10) as xp, \
                 tc.tile_pool(name="rawp", bufs=2) as rawp, \
                 tc.tile_pool(name="tmpa", bufs=2) as tmpa, \
                 tc.tile_pool(name="psA", bufs=1, space="PSUM") as psA, \
                 tc.tile_pool(name="psR", bufs=2, space="PSUM") as psR:
                # weight pair-chunks for DoubleRow: [p, hl, s, r] per n2
                WA = [wap.tile([128, 2 * 2 * RG], FP8, name=f"wa{n}",
                               tag=f"wa{n}")
                      for n in range(NC2)]
                # fp8 3-term: hi@hi + hi@lo + lo@hi (w_hl, x_hl)
                TERMS = ((0, 0), (0, 1), (1, 0))

                def rope_item(tb, j, raw):
                    """PE/DVE tail of rope for comp j of block tb."""
                    ts = slice(tb * 512, (tb + 1) * 512)
                    def emit():
                        rot = psR.tile([128, 512], F32, tag="rot")
                        nc.tensor.matmul(rot[:], PERM[:], raw[:],
                                         start=True, stop=True)
                        t1 = tmpa.tile([128, 512], BF16, tag="t1")
                        nc.vector.tensor_tensor(t1[:], raw[:], COS[:, ts],
                                                mybir.AluOpType.mult)
                        t2 = tmpa.tile([128, 512], BF16, tag="t2")
                        nc.vector.tensor_tensor(t2[:], rot[:], SIN[:, ts],
                                                mybir.AluOpType.mult)
                        nc.vector.tensor_tensor(QQ[j][tb][:], t1[:], t2[:],
                                                mybir.AluOpType.add)
                    return emit

                def v_item(tb, vraw, u):
                    def emit():
                        vt = psR.tile([128, 128], BF16, tag="rot")
                        nc.tensor.transpose(
                            vt[:], vraw[:, u * 128:(u + 1) * 128], IDN[:])
                        nc.scalar.copy(V[tb][:, u * 128:(u + 1) * 128],
                                       vt[:])
                    return emit

                deferred = []
                for tb in range(NT):
                    ts = slice(tb * 512, (tb + 1) * 512)
                    qkv_ps = [psA.tile([128, 512], F32, tag=f"qkv{j}",
                                       name=f"qkv{j}")
                              for j in range(NCOMP)]
                    for n in range(NC2):
                        if tb == 0:
                            for hl in range(2):
                                nc.sync.dma_start(
                                    WA[n][:].rearrange(
                                        "p (hl s r) -> p hl s r",
                                        hl=2, s=2)[:, hl],
                                    d_wa[hl, 256 * n:256 * (n + 1), :]
                                    .rearrange("(s p) r -> p s r", s=2))
                        xt = xp.tile([128, 2 * 2 * 512], FP8, tag="x")
                        for hl in range(2):
                            # split DMA issue: hi plane on the idle Pool
                            # SWDGE queue, lo plane on SP/HWDGE, so neither
                            # generator paces the matmul loop
                            eng = nc.gpsimd if hl == 0 else nc.sync
                            eng.dma_start(
                                xt[:].rearrange("p (hl s t) -> p hl s t",
                                                hl=2, s=2)[:, hl],
                                d_x[hl, 256 * n:256 * (n + 1), ts]
                                .rearrange("(s p) t -> p s t", s=2))
                        Wv = WA[n][:].rearrange("p (hl s r) -> p hl s r",
                                                hl=2, s=2)
                        Xv = xt[:].rearrange("p (hl s t) -> p hl s t",
                                             hl=2, s=2)
                        for j in range(NCOMP):
                            for ti, (wi, xi) in enumerate(TERMS):
                                nc.tensor.matmul(
                                    qkv_ps[j][:],
                                    Wv[:, wi, :, j * HS:(j + 1) * HS],
                                    Xv[:, xi],
                                    start=(n == 0 and ti == 0),
                                    stop=(n == NC2 - 1 and ti == 2),
                                    perf_mode=DR)
                        # spread the previous block's rope/transpose PE
                        # work between this block's GEMM chunks so the PE
                        # queue never drains on the Act/DVE rope chain
                        if deferred and n < len(deferred):
                            deferred[n]()
                        # spread the constant loads through tb0's compute
                        # so they don't contend with x loads at the tb0/tb1
                        # boundary; cos/sin load one t-block slice per tb
                        if tb == 0 and 4 <= n < 8:
                            cdma = [(PERM, d_perm), (IDN, d_idn),
                                    (ONESB, d_ones), (TRI, d_tri)][n - 4]
                            eng = nc.gpsimd if n % 2 == 0 else nc.sync
                            eng.dma_start(cdma[0][:], cdma[1][:])
                        if n == 8:
                            nc.gpsimd.dma_start(COS[:, ts], d_cos[:, ts])
                        if n == 9:
                            nc.sync.dma_start(SIN[:, ts], d_sin[:, ts])
                        if tb == 0 and n == 10:
                            nc.sync.dma_start(ONES32[:], d_ones32[:])

                    deferred = []

                    for j in range(5):  # q0..q3, k get rope
                        raw = rawp.tile([128, 512], BF16, tag=f"raw{j}")
                        nc.scalar.mul(raw[:], qkv_ps[j][:], 1.0 / WSCALE)
                        deferred.append(rope_item(tb, j, raw))

                    # v: transpose [hs, t] -> [t, hs] chunks
                    vraw = rawp.tile([128, 512], BF16, tag="raw5")
                    nc.scalar.mul(vraw[:], qkv_ps[5][:], 1.0 / WSCALE)
                    for u in range(4):
                        deferred.append(v_item(tb, vraw, u))
                for it in deferred:   # flush the last block's rope
                    it()

            # ---------------- Phase B: causal attention ------------------
            with tc.tile_pool(name="wp", bufs=1) as wpp, \
                 tc.tile_pool(name="ptp", bufs=6) as ptp, \
                 tc.tile_pool(name="rcp", bufs=2) as rcp:
                WP = wpp.tile([128, 2 * QPK * C], FP8)   # [p, hl, h, c]
                for hl in range(2):
                    nc.sync.dma_start(
                        WP[:].rearrange("p (hl h c) -> p hl h c",
                                        hl=2, h=QPK)[:, hl],
                        d_wp[hl].rearrange("(h p) c -> p h c", h=QPK))
                # Y split per head-pair so the projection's first pair does
                # not falsely wait on writes for the second pair's heads
                Y8a = wpp.tile([128, 2 * 2 * T], FP8)    # [p, hl, h01, t]
                Y8b = wpp.tile([128, 2 * 2 * T], FP8)    # [p, hl, h23, t]

                with tc.tile_pool(name="psS", bufs=2, space="PSUM") as psS, \
                     tc.tile_pool(name="psY", bufs=2, space="PSUM") as psY, \
                     tc.tile_pool(name="psD", bufs=2, space="PSUM") as psD, \
                     tc.tile_pool(name="psO", bufs=2, space="PSUM") as psO, \
                     tc.tile_pool(name="accp", bufs=2) as accp, \
                     tc.tile_pool(name="outp", bufs=2) as outp:
                    WPv = WP[:].rearrange("p (hl h c) -> p hl h c",
                                          hl=2, h=QPK)
                    CTERMS = ((0, 0), (0, 1), (1, 0))   # (y_hl, w_hl)
                    for b in range(NT):
                        for h in range(QPK):
                            nkt = 4 * (b + 1)
                            y_ps = psY.tile([128, 512], F32, tag="y")
                            d_ps = psD.tile([128, 512], F32, tag="d")
                            # non-diagonal (full-width) P tiles accumulate
                            # on DVE/Pool in f32; only the 4 diagonal tiles
                            # and the two closing partition-sum matmuls use
                            # the PE for the softmax denominator
                            nacc = nkt - 4
                            if nacc > 0:
                                acc_e = accp.tile([128, 512], F32R, tag="ae")
                                acc_o = accp.tile([128, 512], F32R, tag="ao")
                            first_e = first_o = True
                            # software pipeline: issue QK(kt) before
                            # AV/D(kt-1) so PE rolls past the exp latency
                            pend = None
                            for kt in range(nkt):
                                if kt == 2 and pend_dacc is not None:
                                    pend_dacc()
                                    pend_dacc = None
                                r = kt - 4 * b
                                # exact causal narrowing: tile r covers
                                # columns >= r*128; the leading 128-wide
                                # strip gets the triangular mask
                                off = 0 if r < 0 else r * 128
                                s_ps = psS.tile([128, 512], F32, tag="s")
                                nc.tensor.matmul(
                                    s_ps[:, off:],
                                    QQ[4][kt // 4][:, (kt % 4) * 128:
                                                   (kt % 4 + 1) * 128],
                                    QQ[h][b][:, off:],
                                    start=True, stop=True)
                                p_sb = ptp.tile([128, 512], BF16, tag="p")
                                if pend is not None:
                                    kp, offp, pp = pend
                                    nc.tensor.matmul(
                                        y_ps[:, offp:],
                                        V[kp // 4][:, (kp % 4) * 128:
                                                   (kp % 4 + 1) * 128],
                                        pp[:, offp:],
                                        start=(kp == 0), stop=False)
                                    if kp >= nacc:
                                        nc.tensor.matmul(
                                            d_ps[:, offp:], ONESB[:],
                                            pp[:, offp:],
                                            start=(kp == nacc), stop=False)
                                nc.scalar.activation(
                                    p_sb[:, off:], s_ps[:, off:],
                                    mybir.ActivationFunctionType.Exp,
                                    scale=SCALE)
                                if r >= 0:
                                    nc.vector.tensor_tensor(
                                        p_sb[:, off:off + 128],
                                        p_sb[:, off:off + 128],
                                        TRI[:],
                                        mybir.AluOpType.mult)
                                else:
                                    # full-width tile: accumulate for the
                                    # denominator off the PE (Pool is
                                    # ~1.7x slower per op -> gets 1/3)
                                    pool_turn = (kt % 3 == 2)
                                    eng = nc.gpsimd if pool_turn else nc.vector
                                    acc = acc_o if pool_turn else acc_e
                                    first = first_o if pool_turn else first_e
                                    with nc.allow_low_precision(
                                            reason="f32r acc"):
                                        if first:
                                            eng.tensor_copy(acc[:], p_sb[:])
                                        else:
                                            eng.tensor_tensor(
                                                acc[:], acc[:], p_sb[:],
                                                mybir.AluOpType.add)
                                    if pool_turn:
                                        first_o = False
                                    else:
                                        first_e = False
                                pend = (kt, off, p_sb)
                            kp, offp, pp = pend
                            nc.tensor.matmul(
                                y_ps[:, offp:],
                                V[kp // 4][:, (kp % 4) * 128:
                                           (kp % 4 + 1) * 128],
                                pp[:, offp:],
                                start=(kp == 0), stop=True)
                            nc.tensor.matmul(
                                d_ps[:, offp:], ONESB[:], pp[:, offp:],
                                start=(kp == nacc), stop=(nacc == 0))
                            if nacc > 0:
                                nc.tensor.matmul(
                                    d_ps[:], ONES32[:], acc_e[:],
                                    start=False, stop=False)
                                nc.tensor.matmul(
                                    d_ps[:], ONES32[:], acc_o[:],
                                    start=False, stop=True)
                            recip = rcp.tile([128, 512], F32R, tag="r")
                            with nc.allow_low_precision(
                                    reason="float32r is float32-width"):
                                nc.vector.reciprocal(recip[:], d_ps[:])
                            y_bf = rcp.tile([128, 512], BF16, tag="ybf")
                            nc.vector.tensor_tensor(
                                y_bf[:], y_ps[:], recip[:],
                                mybir.AluOpType.mult)
                            bs = slice(b * 512, (b + 1) * 512)
                            Yp = (Y8a if h < 2 else Y8b)[:].rearrange(
                                "p (hl h t) -> p hl h t", hl=2, h=2)
                            nc.scalar.copy(Yp[:, 0, h % 2, bs], y_bf[:])
                            with nc.allow_low_precision(
                                    reason="fp8 residual split"):
                                nc.vector.tensor_tensor(
                                    Yp[:, 1, h % 2, bs], y_bf[:],
                                    Yp[:, 0, h % 2, bs],
                                    mybir.AluOpType.subtract)

                        # ---- output projection for this t-block ----
                        # runs as soon as all 4 heads of block b are done,
                        # filling PE while the next block is Act-bound
                        for tt in range(4 * b, 4 * (b + 1)):
                            o_sb = outp.tile([128, C], BF16, tag="o")
                            for cb in range(C // 512):
                                o_ps = psO.tile([128, 512], F32, tag="o")
                                for hp in range(QPK // 2):
                                    Ypr = (Y8a if hp == 0 else Y8b)[:]\
                                        .rearrange("p (hl h t) -> p hl h t",
                                                   hl=2, h=2)
                                    hsl = slice(2 * hp, 2 * hp + 2)
                                    tsl = slice(tt * 128, (tt + 1) * 128)
                                    csl = slice(cb * 512, (cb + 1) * 512)
                                    for ti, (yi, wi) in enumerate(CTERMS):
                                        nc.tensor.matmul(
                                            o_ps[:],
                                            Ypr[:, yi, :, tsl],
                                            WPv[:, wi, hsl, csl],
                                            start=(hp == 0 and ti == 0),
                                            stop=(hp == 1 and ti == 2),
                                            perf_mode=DR)
                                # alternate Act/DVE for PSUM->SBUF copies
                                if cb % 2 == 0:
                                    nc.scalar.copy(
                                        o_sb[:, cb * 512:(cb + 1) * 512],
                                        o_ps[:])
                                else:
                                    nc.vector.tensor_copy(
                                        o_sb[:, cb * 512:(cb + 1) * 512],
                                        o_ps[:])
                                if cb % 2 == 1:
                                    cs0 = (cb - 1) * 512
                                    nc.sync.dma_start(
                                        d_out[tt * 128:(tt + 1) * 128,
                                              cs0:cs0 + 1024],
                                        o_sb[:, cs0:cs0 + 1024])
    nc.compile()
    return nc


def _split8(a):
    """fp8 hi/lo split: a ~ unit sigma -> [2, ...] e4m3 stack."""
    hi = a.astype(NP8)
    lo = (a - hi.astype(np.float32)).astype(NP8)
    return np.ascontiguousarray(np.stack([hi, lo]))


def _host_inputs(x, cos, sin, w_attn, w_proj):
    """Build per-core input maps (host-side shard + transpose prep)."""
    f = np.float32
    x8 = _split8(np.ascontiguousarray(x.reshape(T, C).T))            # [2, C, T]
    cost = np.ascontiguousarray(cos.T).astype(NPBF16)                # [HS, T]
    sgn = np.ones((HS, 1), f)
    sgn[:HS // 2] = -1.0
    sint = np.ascontiguousarray(sin.T * sgn).astype(NPBF16)          # signed sin
    # rot(x)=P@x in [d,t] layout; matmul computes lhsT.T @ rhs -> lhsT = P.T
    P = np.zeros((HS, HS), f)
    for i in range(HS // 2):
        P[i, i + HS // 2] = 1.0
        P[i + HS // 2, i] = 1.0
    perm = np.ascontiguousarray(P.T).astype(NPBF16)
    idn = np.eye(128, dtype=f).astype(NPBF16)
    ones = np.ones((128, 128), f).astype(NPBF16)
    # triangular strip mask: keep iff col >= row
    iidx = np.arange(128)
    tri = (iidx[None, :] >= iidx[:, None]).astype(f).astype(NPBF16)

    maps = []
    for g in range(N_CORES):
        wa8 = _split8(WSCALE *
                      np.ascontiguousarray(w_attn[g * RG:(g + 1) * RG, :].T))
        wp8 = _split8(WSCALE *
                      np.ascontiguousarray(w_proj[:, g * OG:(g + 1) * OG].T))
        maps.append({
            "x8": x8, "wa8": wa8, "wp8": wp8, "cost": cost, "sint": sint,
            "tri": tri, "perm": perm, "idn": idn, "ones": ones,
            "ones32": np.ones((128, 128), f),
        })
    return maps


_PROGRAM = None


def kernel(x, cos, sin, w_attn, w_proj):
    global _PROGRAM
    if _PROGRAM is None:
        _PROGRAM = _build_program()
    maps = _host_inputs(np.asarray(x), np.asarray(cos), np.asarray(sin),
                        np.asarray(w_attn), np.asarray(w_proj))
    res = bass_utils.run_bass_kernel_spmd(_PROGRAM, maps, list(range(N_CORES)))
    out = np.zeros((T, C), np.float32)
    for g in range(N_CORES):
        out += np.asarray(res.results[g]["out"], dtype=np.float32)
    return (out / WSCALE).reshape(1, T, C)


# revision 73
# speedup vs baseline: 1.1522x; 1.0760x over previous
"""Trainium2 Bass kernel for CausalSelfAttention (B=1, T=2048, C=4096,
32 heads / 8 query groups / head_size 128, full-dim RoPE, GQA).

Sharding: tensor-parallel over the 8 query groups. Core g owns w_attn rows
[g*768:(g+1)*768] (4 q heads + 1 k + 1 v) and w_proj columns
[g*512:(g+1)*512]; x is replicated. Each core returns a partial projection
output [2048, 4096] (bf16); the host sums the 8 partials (the all-reduce).

Final architecture (418534ns baseline -> 317475ns):
- qkv + output projections run as fp8e4 (e4m3) DoubleRow matmuls (0.5
  cycles/row, 2x contraction per instr) with 3-term hi/lo error
  compensation (xh@wh + xl@wh + xh@wl); weights pre-scaled x64 to escape
  e4m3's subnormal range, descaled in the PSUM-read copies / host gather.
- attention in bf16 with exact causal narrowing; scores kept transposed
  ([k, q]) so AV accumulates in PSUM; no max-subtraction needed (fp32/bf16
  exp range suffices; fp8 P is impossible: diagonal-dominant scores
  overflow e4m3).
- softmax denominator: full-width P tiles accumulate elementwise on
  DVE (2/3) + Pool (1/3) in f32; diagonal tiles and two closing all-ones
  partition-sum matmuls on PE; close + normalize deferred into the next
  head's QK stream.
- scheduling: software-pipelined QK/AV, RoPE tails deferred into the next
  block's GEMM chunks, per-block projection deferred one block so the next
  block's attention hides the epilogue, split-engine DMA issue (Pool SWDGE
  + SP HWDGE), constants spread through tb0, split output stores.
"""

import os
import sys

for _p in ("/opt/trn_rl_repo", "/root/.axon_site/_ro/trn_rl_repo"):
    if os.path.isdir(_p) and _p not in sys.path:
        sys.path.insert(0, _p)

import numpy as np
import ml_dtypes

import concourse.bass as bass
import concourse.mybir as mybir
import concourse.tile as tile
from concourse import bacc, bass_utils

N_CORES = 8
T = 2048
C = 4096
HS = 128
N_HEAD = 32
G = 8                      # query groups == cores
QPK = 4                    # q heads per group
NCOMP = QPK + 2            # q0..q3, k, v
RG = NCOMP * HS            # 768 w_attn rows per group
OG = QPK * HS              # 512 proj-input cols per group
NT = T // 512              # 4 blocks of 512 along t
NC = C // 128              # 32 contraction chunks
NQ = C // 512              # 8 contraction quads
SCALE = 1.0 / np.sqrt(float(HS))

F32 = mybir.dt.float32
F32R = mybir.dt.float32r
BF16 = mybir.dt.bfloat16
FP8 = mybir.dt.float8e4
NPBF16 = ml_dtypes.bfloat16
NP8 = ml_dtypes.float8_e4m3
DR = mybir.MatmulPerfMode.DoubleRow
NC2 = C // 256             # 16 double-row pair chunks
WSCALE = 64.0              # weight pre-scale so fp8 operands are ~unit sigma


def _build_program():
    nc = bacc.Bacc(trn_type="TRN2", target_bir_lowering=False, debug=False,
                   num_devices=N_CORES)

    d_x = nc.dram_tensor("x8", [2, C, T], FP8, kind="ExternalInput").ap()
    d_wa = nc.dram_tensor("wa8", [2, C, RG], FP8, kind="ExternalInput").ap()
    d_wp = nc.dram_tensor("wp8", [2, OG, C], FP8, kind="ExternalInput").ap()
    d_cos = nc.dram_tensor("cost", [HS, T], BF16, kind="ExternalInput").ap()
    d_sin = nc.dram_tensor("sint", [HS, T], BF16, kind="ExternalInput").ap()
    d_tri = nc.dram_tensor("tri", [128, 128], BF16, kind="ExternalInput").ap()
    d_perm = nc.dram_tensor("perm", [128, 128], BF16, kind="ExternalInput").ap()
    d_idn = nc.dram_tensor("idn", [128, 128], BF16, kind="ExternalInput").ap()
    d_ones = nc.dram_tensor("ones", [128, 128], BF16, kind="ExternalInput").ap()
    d_ones32 = nc.dram_tensor("ones32", [128, 128], F32R,
                              kind="ExternalInput").ap()
    d_out = nc.dram_tensor("out", [T, C], BF16, kind="ExternalOutput").ap()

    with tile.TileContext(nc) as tc:
        with tc.tile_pool(name="glob", bufs=1) as glob:
            # roped q0..q3 / k, one tile per (comp, t-block): [hs=128, 512]
            QQ = [[glob.tile([128, 512], BF16, name=f"qq{j}_{tb}",
                             tag=f"qq{j}_{tb}")
                   for tb in range(NT)] for j in range(5)]
            # V in [t, hs] layout, one tile per t-block: col u = t-chunk
            V = [glob.tile([128, 512], BF16, name=f"v{tb}", tag=f"v{tb}")
                 for tb in range(NT)]
            ONESB = glob.tile([128, 128], BF16)
            ONES32 = glob.tile([128, 128], F32R)
            PERM = glob.tile([128, 128], BF16)
            IDN = glob.tile([128, 128], BF16)
            TRI = glob.tile([128, 128], BF16)
            COS = glob.tile([128, T], BF16)
            SIN = glob.tile([128, T], BF16)

            # ---------------- Phase A: qkv projection + rope -------------
            with tc.tile_pool(name="wa", bufs=1) as wap, \
                 tc.tile_pool(name="xp", bufs=10) as xp, \
                 tc.tile_pool(name="rawp", bufs=2) as rawp, \
                 tc.tile_pool(name="tmpa", bufs=2) as tmpa, \
                 tc.tile_pool(name="psA", bufs=1, space="PSUM") as psA, \
                 tc.tile_pool(name="psR", bufs=2, space="PSUM") as psR:
                # weight pair-chunks for DoubleRow: [p, hl, s, r] per n2
                WA = [wap.tile([128, 2 * 2 * RG], FP8, name=f"wa{n}",
                               tag=f"wa{n}")
                      for n in range(NC2)]
                # fp8 3-term: hi@hi + hi@lo + lo@hi (w_hl, x_hl)
                TERMS = ((0, 0), (0, 1), (1, 0))

                def rope_item(tb, j, raw):
                    """PE/DVE tail of rope for comp j of block tb."""
                    ts = slice(tb * 512, (tb + 1) * 512)
                    def emit():
                        rot = psR.tile([128, 512], F32, tag="rot")
                        nc.tensor.matmul(rot[:], PERM[:], raw[:],
                                         start=True, stop=True)
                        t1 = tmpa.tile([128, 512], BF16, tag="t1")
                        nc.vector.tensor_tensor(t1[:], raw[:], COS[:, ts],
                                                mybir.AluOpType.mult)
                        t2 = tmpa.tile([128, 512], BF16, tag="t2")
                        nc.vector.tensor_tensor(t2[:], rot[:], SIN[:, ts],
                                                mybir.AluOpType.mult)
                        nc.vector.tensor_tensor(QQ[j][tb][:], t1[:], t2[:],
                                                mybir.AluOpType.add)
                    return emit

                def v_item(tb, vraw, u):
                    def emit():
                        vt = psR.tile([128, 128], BF16, tag="rot")
                        nc.tensor.transpose(
                            vt[:], vraw[:, u * 128:(u + 1) * 128], IDN[:])
                        nc.scalar.copy(V[tb][:, u * 128:(u + 1) * 128],
                                       vt[:])
                    return emit

                deferred = []
                for tb in range(NT):
                    ts = slice(tb * 512, (tb + 1) * 512)
                    qkv_ps = [psA.tile([128, 512], F32, tag=f"qkv{j}",
                                       name=f"qkv{j}")
                              for j in range(NCOMP)]
                    for n in range(NC2):
                        if tb == 0:
                            for hl in range(2):
                                nc.sync.dma_start(
                                    WA[n][:].rearrange(
                                        "p (hl s r) -> p hl s r",
                                        hl=2, s=2)[:, hl],
                                    d_wa[hl, 256 * n:256 * (n + 1), :]
                                    .rearrange("(s p) r -> p s r", s=2))
                        xt = xp.tile([128, 2 * 2 * 512], FP8, tag="x")
                        for hl in range(2):
                            # split DMA issue: hi plane on the idle Pool
                            # SWDGE queue, lo plane on SP/HWDGE, so neither
                            # generator paces the matmul loop
                            eng = nc.gpsimd if hl == 0 else nc.sync
                            eng.dma_start(
                                xt[:].rearrange("p (hl s t) -> p hl s t",
                                                hl=2, s=2)[:, hl],
                                d_x[hl, 256 * n:256 * (n + 1), ts]
                                .rearrange("(s p) t -> p s t", s=2))
                        Wv = WA[n][:].rearrange("p (hl s r) -> p hl s r",
                                                hl=2, s=2)
                        Xv = xt[:].rearrange("p (hl s t) -> p hl s t",
                                             hl=2, s=2)
                        for j in range(NCOMP):
                            for ti, (wi, xi) in enumerate(TERMS):
                                nc.tensor.matmul(
                                    qkv_ps[j][:],
                                    Wv[:, wi, :, j * HS:(j + 1) * HS],
                                    Xv[:, xi],
                                    start=(n == 0 and ti == 0),
                                    stop=(n == NC2 - 1 and ti == 2),
                                    perf_mode=DR)
                        # spread the previous block's rope/transpose PE
                        # work between this block's GEMM chunks so the PE
                        # queue never drains on the Act/DVE rope chain
                        if deferred and n < len(deferred):
                            deferred[n]()
                        # spread the constant loads through tb0's compute
                        # so they don't contend with x loads at the tb0/tb1
                        # boundary; cos/sin load one t-block slice per tb
                        if tb == 0 and 4 <= n < 8:
                            cdma = [(PERM, d_perm), (IDN, d_idn),
                                    (ONESB, d_ones), (TRI, d_tri)][n - 4]
                            eng = nc.gpsimd if n % 2 == 0 else nc.sync
                            eng.dma_start(cdma[0][:], cdma[1][:])
                        if n == 8:
                            nc.gpsimd.dma_start(COS[:, ts], d_cos[:, ts])
                        if n == 9:
                            nc.sync.dma_start(SIN[:, ts], d_sin[:, ts])
                        if tb == 0 and n == 10:
                            nc.sync.dma_start(ONES32[:], d_ones32[:])

                    deferred = []

                    for j in range(5):  # q0..q3, k get rope
                        raw = rawp.tile([128, 512], BF16, tag=f"raw{j}")
                        # alternate Act/DVE so the six descale copies drain
                        # in ~half the serial latency (frees PSUM banks and
                        # unblocks the deferred rope sooner)
                        if j % 2 == 0:
                            nc.scalar.mul(raw[:], qkv_ps[j][:], 1.0 / WSCALE)
                        else:
                            nc.vector.tensor_scalar_mul(
                                raw[:], qkv_ps[j][:], 1.0 / WSCALE)
                        deferred.append(rope_item(tb, j, raw))

                    # v: transpose [hs, t] -> [t, hs] chunks
                    vraw = rawp.tile([128, 512], BF16, tag="raw5")
                    nc.vector.tensor_scalar_mul(
                        vraw[:], qkv_ps[5][:], 1.0 / WSCALE)
                    for u in range(4):
                        deferred.append(v_item(tb, vraw, u))
                for it in deferred:   # flush the last block's rope
                    it()

            # ---------------- Phase B: causal attention ------------------
            with tc.tile_pool(name="wp", bufs=1) as wpp, \
                 tc.tile_pool(name="ptp", bufs=8) as ptp, \
                 tc.tile_pool(name="rcp", bufs=3) as rcp:
                WP = wpp.tile([128, 2 * QPK * C], FP8)   # [p, hl, h, c]
                for hl in range(2):
                    nc.sync.dma_start(
                        WP[:].rearrange("p (hl h c) -> p hl h c",
                                        hl=2, h=QPK)[:, hl],
                        d_wp[hl].rearrange("(h p) c -> p h c", h=QPK))
                # Y split per head-pair so the projection's first pair does
                # not falsely wait on writes for the second pair's heads
                Y8a = wpp.tile([128, 2 * 2 * T], FP8)    # [p, hl, h01, t]
                Y8b = wpp.tile([128, 2 * 2 * T], FP8)    # [p, hl, h23, t]

                with tc.tile_pool(name="psS", bufs=2, space="PSUM") as psS, \
                     tc.tile_pool(name="psY", bufs=2, space="PSUM") as psY, \
                     tc.tile_pool(name="psD", bufs=2, space="PSUM") as psD, \
                     tc.tile_pool(name="psO", bufs=2, space="PSUM") as psO, \
                     tc.tile_pool(name="accp", bufs=3) as accp, \
                     tc.tile_pool(name="outp", bufs=2) as outp:
                    WPv = WP[:].rearrange("p (hl h c) -> p hl h c",
                                          hl=2, h=QPK)
                    CTERMS = ((0, 0), (0, 1), (1, 0))   # (y_hl, w_hl)
                    pend_dacc = None
                    for b in range(NT):
                        for h in range(QPK):
                            nkt = 4 * (b + 1)
                            y_ps = psY.tile([128, 512], F32, tag="y")
                            d_ps = psD.tile([128, 512], F32, tag="d")
                            # non-diagonal (full-width) P tiles accumulate
                            # on DVE/Pool in f32; only the 4 diagonal tiles
                            # and the two closing partition-sum matmuls use
                            # the PE for the softmax denominator
                            nacc = nkt - 4
                            acc_e = acc_o = None
                            if nacc > 0:
                                acc_e = accp.tile([128, 512], F32R, tag="ae")
                                acc_o = accp.tile([128, 512], F32R, tag="ao")
                            first_e = first_o = True
                            # software pipeline: issue QK(kt) before
                            # AV/D(kt-1) so PE rolls past the exp latency
                            pend = None
                            for kt in range(nkt):
                                if kt == 2 and pend_dacc is not None:
                                    pend_dacc()
                                    pend_dacc = None
                                r = kt - 4 * b
                                # exact causal narrowing: tile r covers
                                # columns >= r*128; the leading 128-wide
                                # strip gets the triangular mask
                                off = 0 if r < 0 else r * 128
                                s_ps = psS.tile([128, 512], F32, tag="s")
                                nc.tensor.matmul(
                                    s_ps[:, off:],
                                    QQ[4][kt // 4][:, (kt % 4) * 128:
                                                   (kt % 4 + 1) * 128],
                                    QQ[h][b][:, off:],
                                    start=True, stop=True)
                                p_sb = ptp.tile([128, 512], BF16, tag="p")
                                if pend is not None:
                                    kp, offp, pp = pend
                                    nc.tensor.matmul(
                                        y_ps[:, offp:],
                                        V[kp // 4][:, (kp % 4) * 128:
                                                   (kp % 4 + 1) * 128],
                                        pp[:, offp:],
                                        start=(kp == 0), stop=False)
                                    if kp >= nacc:
                                        nc.tensor.matmul(
                                            d_ps[:, offp:], ONESB[:],
                                            pp[:, offp:],
                                            start=(kp == nacc), stop=False)
                                nc.scalar.activation(
                                    p_sb[:, off:], s_ps[:, off:],
                                    mybir.ActivationFunctionType.Exp,
                                    scale=SCALE)
                                if r >= 0:
                                    nc.vector.tensor_tensor(
                                        p_sb[:, off:off + 128],
                                        p_sb[:, off:off + 128],
                                        TRI[:],
                                        mybir.AluOpType.mult)
                                else:
                                    # full-width tile: accumulate for the
                                    # denominator off the PE (Pool is
                                    # ~1.7x slower per op -> gets 1/3)
                                    pool_turn = (kt % 3 == 2)
                                    eng = nc.gpsimd if pool_turn else nc.vector
                                    acc = acc_o if pool_turn else acc_e
                                    first = first_o if pool_turn else first_e
                                    with nc.allow_low_precision(
                                            reason="f32r acc"):
                                        if first:
                                            eng.tensor_copy(acc[:], p_sb[:])
                                        else:
                                            eng.tensor_tensor(
                                                acc[:], acc[:], p_sb[:],
                                                mybir.AluOpType.add)
                                    if pool_turn:
                                        first_o = False
                                    else:
                                        first_e = False
                                pend = (kt, off, p_sb)
                            kp, offp, pp = pend
                            nc.tensor.matmul(
                                y_ps[:, offp:],
                                V[kp // 4][:, (kp % 4) * 128:
                                           (kp % 4 + 1) * 128],
                                pp[:, offp:],
                                start=(kp == 0), stop=True)
                            nc.tensor.matmul(
                                d_ps[:, offp:], ONESB[:], pp[:, offp:],
                                start=(kp == nacc), stop=(nacc == 0))

                            def dacc_close(b=b, h=h, nacc=nacc,
                                           acc_e=(acc_e if nacc else None),
                                           acc_o=(acc_o if nacc else None),
                                           d_ps=d_ps, y_ps=y_ps):
                                if nacc > 0:
                                    nc.tensor.matmul(
                                        d_ps[:], ONES32[:], acc_e[:],
                                        start=False, stop=False)
                                    nc.tensor.matmul(
                                        d_ps[:], ONES32[:], acc_o[:],
                                        start=False, stop=True)
                                recip = rcp.tile([128, 512], F32R, tag="r")
                                with nc.allow_low_precision(
                                        reason="float32r is float32-width"):
                                    nc.vector.reciprocal(recip[:], d_ps[:])
                                y_bf = rcp.tile([128, 512], BF16, tag="ybf")
                                nc.vector.tensor_tensor(
                                    y_bf[:], y_ps[:], recip[:],
                                    mybir.AluOpType.mult)
                                bs = slice(b * 512, (b + 1) * 512)
                                Yp = (Y8a if h < 2 else Y8b)[:].rearrange(
                                    "p (hl h t) -> p hl h t", hl=2, h=2)
                                nc.scalar.copy(Yp[:, 0, h % 2, bs], y_bf[:])
                                with nc.allow_low_precision(
                                        reason="fp8 residual split"):
                                    nc.vector.tensor_tensor(
                                        Yp[:, 1, h % 2, bs], y_bf[:],
                                        Yp[:, 0, h % 2, bs],
                                        mybir.AluOpType.subtract)
                            # defer the denominator close + normalize into
                            # the next head's QK stream so the PE doesn't
                            # idle waiting for the lagging DVE/Pool adds
                            pend_dacc = dacc_close

                        if pend_dacc is not None:
                            pend_dacc()
                            pend_dacc = None

                    # ---- output projection, deferred one block ----
                    # proj(pb) is emitted after attention of block pb+1, so
                    # the next block's GEMMs hide the last head's epilogue
                    # chain; the final block's projection follows the
                    # second-to-last's with all dependencies long satisfied
                    for pb in ([b - 1, b] if b == NT - 1 else [b - 1]):
                        if pb < 0:
                            continue
                        for tt in range(4 * pb, 4 * (pb + 1)):
                            o_sb = outp.tile([128, C], BF16, tag="o")
                            for cb in range(C // 512):
                                o_ps = psO.tile([128, 512], F32, tag="o")
                                for hp in range(QPK // 2):
                                    Ypr = (Y8a if hp == 0 else Y8b)[:]\
                                        .rearrange("p (hl h t) -> p hl h t",
                                                   hl=2, h=2)
                                    hsl = slice(2 * hp, 2 * hp + 2)
                                    tsl = slice(tt * 128, (tt + 1) * 128)
                                    csl = slice(cb * 512, (cb + 1) * 512)
                                    for ti, (yi, wi) in enumerate(CTERMS):
                                        nc.tensor.matmul(
                                            o_ps[:],
                                            Ypr[:, yi, :, tsl],
                                            WPv[:, wi, hsl, csl],
                                            start=(hp == 0 and ti == 0),
                                            stop=(hp == 1 and ti == 2),
                                            perf_mode=DR)
                                # alternate Act/DVE for PSUM->SBUF copies
                                if cb % 2 == 0:
                                    nc.scalar.copy(
                                        o_sb[:, cb * 512:(cb + 1) * 512],
                                        o_ps[:])
                                else:
                                    nc.vector.tensor_copy(
                                        o_sb[:, cb * 512:(cb + 1) * 512],
                                        o_ps[:])
                                if cb % 2 == 1:
                                    cs0 = (cb - 1) * 512
                                    nc.sync.dma_start(
                                        d_out[tt * 128:(tt + 1) * 128,
                                              cs0:cs0 + 1024],
                                        o_sb[:, cs0:cs0 + 1024])
    nc.compile()
    return nc


def _split8(a):
    """fp8 hi/lo split: a ~ unit sigma -> [2, ...] e4m3 stack."""
    hi = a.astype(NP8)
    lo = (a - hi.astype(np.float32)).astype(NP8)
    return np.ascontiguousarray(np.stack([hi, lo]))


def _host_inputs(x, cos, sin, w_attn, w_proj):
    """Build per-core input maps (host-side shard + transpose prep)."""
    f = np.float32
    x8 = _split8(np.ascontiguousarray(x.reshape(T, C).T))            # [2, C, T]
    cost = np.ascontiguousarray(cos.T).astype(NPBF16)                # [HS, T]
    sgn = np.ones((HS, 1), f)
    sgn[:HS // 2] = -1.0
    sint = np.ascontiguousarray(sin.T * sgn).astype(NPBF16)          # signed sin
    # rot(x)=P@x in [d,t] layout; matmul computes lhsT.T @ rhs -> lhsT = P.T
    P = np.zeros((HS, HS), f)
    for i in range(HS // 2):
        P[i, i + HS // 2] = 1.0
        P[i + HS // 2, i] = 1.0
    perm = np.ascontiguousarray(P.T).astype(NPBF16)
    idn = np.eye(128, dtype=f).astype(NPBF16)
    ones = np.ones((128, 128), f).astype(NPBF16)
    # triangular strip mask: keep iff col >= row
    iidx = np.arange(128)
    tri = (iidx[None, :] >= iidx[:, None]).astype(f).astype(NPBF16)

    maps = []
    for g in range(N_CORES):
        wa8 = _split8(WSCALE *
                      np.ascontiguousarray(w_attn[g * RG:(g + 1) * RG, :].T))
        wp8 = _split8(WSCALE *
                      np.ascontiguousarray(w_proj[:, g * OG:(g + 1) * OG].T))
        maps.append({
            "x8": x8, "wa8": wa8, "wp8": wp8, "cost": cost, "sint": sint,
            "tri": tri, "perm": perm, "idn": idn, "ones": ones,
            "ones32": np.ones((128, 128), f),
        })
    return maps


_PROGRAM = None


def kernel(x, cos, sin, w_attn, w_proj):
    global _PROGRAM
    if _PROGRAM is None:
        _PROGRAM = _build_program()
    maps = _host_inputs(np.asarray(x), np.asarray(cos), np.asarray(sin),
                        np.asarray(w_attn), np.asarray(w_proj))
    res = bass_utils.run_bass_kernel_spmd(_PROGRAM, maps, list(range(N_CORES)))
    out = np.zeros((T, C), np.float32)
    for g in range(N_CORES):
        out += np.asarray(res.results[g]["out"], dtype=np.float32)
    return (out / WSCALE).reshape(1, T, C)


# revision 80
# speedup vs baseline: 1.1945x; 1.0367x over previous
"""Trainium2 Bass kernel for CausalSelfAttention (B=1, T=2048, C=4096,
32 heads / 8 query groups / head_size 128, full-dim RoPE, GQA).

Sharding: tensor-parallel over the 8 query groups. Core g owns w_attn rows
[g*768:(g+1)*768] (4 q heads + 1 k + 1 v) and w_proj columns
[g*512:(g+1)*512]; x is replicated. Each core returns a partial projection
output [2048, 4096] (bf16); the host sums the 8 partials (the all-reduce).

Final architecture (418534ns baseline -> 317475ns):
- qkv + output projections run as fp8e4 (e4m3) DoubleRow matmuls (0.5
  cycles/row, 2x contraction per instr) with 3-term hi/lo error
  compensation (xh@wh + xl@wh + xh@wl); weights pre-scaled x64 to escape
  e4m3's subnormal range, descaled in the PSUM-read copies / host gather.
- attention in bf16 with exact causal narrowing; scores kept transposed
  ([k, q]) so AV accumulates in PSUM; no max-subtraction needed (fp32/bf16
  exp range suffices; fp8 P is impossible: diagonal-dominant scores
  overflow e4m3).
- softmax denominator: full-width P tiles accumulate elementwise on
  DVE (2/3) + Pool (1/3) in f32; diagonal tiles and two closing all-ones
  partition-sum matmuls on PE; close + normalize deferred into the next
  head's QK stream.
- scheduling: software-pipelined QK/AV, RoPE tails deferred into the next
  block's GEMM chunks, per-block projection deferred one block so the next
  block's attention hides the epilogue, split-engine DMA issue (Pool SWDGE
  + SP HWDGE), constants spread through tb0, split output stores.
"""

import os
import sys

for _p in ("/opt/trn_rl_repo", "/root/.axon_site/_ro/trn_rl_repo"):
    if os.path.isdir(_p) and _p not in sys.path:
        sys.path.insert(0, _p)

import numpy as np
import ml_dtypes

import concourse.bass as bass
import concourse.mybir as mybir
import concourse.tile as tile
from concourse import bacc, bass_utils

N_CORES = 8
T = 2048
C = 4096
HS = 128
N_HEAD = 32
G = 8                      # query groups == cores
QPK = 4                    # q heads per group
NCOMP = QPK + 2            # q0..q3, k, v
RG = NCOMP * HS            # 768 w_attn rows per group
OG = QPK * HS              # 512 proj-input cols per group
NT = T // 512              # 4 blocks of 512 along t
NC = C // 128              # 32 contraction chunks
NQ = C // 512              # 8 contraction quads
SCALE = 1.0 / np.sqrt(float(HS))

F32 = mybir.dt.float32
F32R = mybir.dt.float32r
BF16 = mybir.dt.bfloat16
FP8 = mybir.dt.float8e4
NPBF16 = ml_dtypes.bfloat16
NP8 = ml_dtypes.float8_e4m3
DR = mybir.MatmulPerfMode.DoubleRow
NC2 = C // 256             # 16 double-row pair chunks
WSCALE = 64.0              # weight pre-scale so fp8 operands are ~unit sigma


def _build_program():
    nc = bacc.Bacc(trn_type="TRN2", target_bir_lowering=False, debug=False,
                   num_devices=N_CORES)

    d_x = nc.dram_tensor("x8", [2, C, T], FP8, kind="ExternalInput").ap()
    d_wa = nc.dram_tensor("wa8", [2, C, RG], FP8, kind="ExternalInput").ap()
    d_wp = nc.dram_tensor("wp8", [2, OG, C], FP8, kind="ExternalInput").ap()
    d_cos = nc.dram_tensor("cost", [HS, T], BF16, kind="ExternalInput").ap()
    d_sin = nc.dram_tensor("sint", [HS, T], BF16, kind="ExternalInput").ap()
    d_tri = nc.dram_tensor("tri", [128, 128], BF16, kind="ExternalInput").ap()
    d_perm = nc.dram_tensor("perm", [128, 128], BF16, kind="ExternalInput").ap()
    d_idn = nc.dram_tensor("idn", [128, 128], BF16, kind="ExternalInput").ap()
    d_ones = nc.dram_tensor("ones", [128, 128], BF16, kind="ExternalInput").ap()
    d_ones32 = nc.dram_tensor("ones32", [128, 128], F32R,
                              kind="ExternalInput").ap()
    d_out = nc.dram_tensor("out", [T, C], BF16, kind="ExternalOutput").ap()

    with tile.TileContext(nc) as tc:
        with tc.tile_pool(name="glob", bufs=1) as glob:
            # roped q0..q3 / k, one tile per (comp, t-block): [hs=128, 512]
            QQ = [[glob.tile([128, 512], BF16, name=f"qq{j}_{tb}",
                             tag=f"qq{j}_{tb}")
                   for tb in range(NT)] for j in range(5)]
            # V in [t, hs] layout, one tile per t-block: col u = t-chunk
            V = [glob.tile([128, 512], BF16, name=f"v{tb}", tag=f"v{tb}")
                 for tb in range(NT)]
            ONESB = glob.tile([128, 128], BF16)
            ONES32 = glob.tile([128, 128], F32R)
            PERM = glob.tile([128, 128], BF16)
            IDN = glob.tile([128, 128], BF16)
            TRI = glob.tile([128, 128], BF16)
            COS = glob.tile([128, T], BF16)
            SIN = glob.tile([128, T], BF16)

            # ---------------- Phase A: qkv projection + rope -------------
            with tc.tile_pool(name="wa", bufs=1) as wap, \
                 tc.tile_pool(name="xp", bufs=10) as xp, \
                 tc.tile_pool(name="rawp", bufs=2) as rawp, \
                 tc.tile_pool(name="tmpa", bufs=2) as tmpa, \
                 tc.tile_pool(name="psA", bufs=1, space="PSUM") as psA, \
                 tc.tile_pool(name="psR", bufs=2, space="PSUM") as psR:
                # weight pair-chunks for DoubleRow: [p, hl, s, r] per n2
                WA = [wap.tile([128, 2 * 2 * RG], FP8, name=f"wa{n}",
                               tag=f"wa{n}")
                      for n in range(NC2)]
                # fp8 3-term: hi@hi + hi@lo + lo@hi (w_hl, x_hl)
                TERMS = ((0, 0), (0, 1), (1, 0))

                def rope_item(tb, j, raw):
                    """PE/DVE tail of rope for comp j of block tb."""
                    ts = slice(tb * 512, (tb + 1) * 512)
                    def emit():
                        rot = psR.tile([128, 512], F32, tag="rot")
                        nc.tensor.matmul(rot[:], PERM[:], raw[:],
                                         start=True, stop=True)
                        t1 = tmpa.tile([128, 512], BF16, tag="t1")
                        nc.vector.tensor_tensor(t1[:], raw[:], COS[:, ts],
                                                mybir.AluOpType.mult)
                        t2 = tmpa.tile([128, 512], BF16, tag="t2")
                        nc.vector.tensor_tensor(t2[:], rot[:], SIN[:, ts],
                                                mybir.AluOpType.mult)
                        nc.vector.tensor_tensor(QQ[j][tb][:], t1[:], t2[:],
                                                mybir.AluOpType.add)
                    return emit

                def v_item(tb, vraw, u):
                    def emit():
                        vt = psR.tile([128, 128], BF16, tag="rot")
                        nc.tensor.transpose(
                            vt[:], vraw[:, u * 128:(u + 1) * 128], IDN[:])
                        nc.scalar.copy(V[tb][:, u * 128:(u + 1) * 128],
                                       vt[:])
                    return emit

                deferred = []
                for tb in range(NT):
                    ts = slice(tb * 512, (tb + 1) * 512)
                    qkv_ps = [psA.tile([128, 512], F32, tag=f"qkv{j}",
                                       name=f"qkv{j}")
                              for j in range(NCOMP)]
                    for n in range(NC2):
                        if tb == 0:
                            for hl in range(2):
                                nc.sync.dma_start(
                                    WA[n][:].rearrange(
                                        "p (hl s r) -> p hl s r",
                                        hl=2, s=2)[:, hl],
                                    d_wa[hl, 256 * n:256 * (n + 1), :]
                                    .rearrange("(s p) r -> p s r", s=2))
                        xt = xp.tile([128, 2 * 2 * 512], FP8, tag="x")
                        for hl in range(2):
                            # split DMA issue: hi plane on the idle Pool
                            # SWDGE queue, lo plane on SP/HWDGE, so neither
                            # generator paces the matmul loop
                            eng = nc.gpsimd if hl == 0 else nc.sync
                            eng.dma_start(
                                xt[:].rearrange("p (hl s t) -> p hl s t",
                                                hl=2, s=2)[:, hl],
                                d_x[hl, 256 * n:256 * (n + 1), ts]
                                .rearrange("(s p) t -> p s t", s=2))
                        Wv = WA[n][:].rearrange("p (hl s r) -> p hl s r",
                                                hl=2, s=2)
                        Xv = xt[:].rearrange("p (hl s t) -> p hl s t",
                                             hl=2, s=2)
                        for j in range(NCOMP):
                            # q0/q1 skip the w-lo term: a q-side 2% weight
                            # quant error only perturbs one head's scores
                            # (measured 7.5e-3/head vs the 2e-2 gate); k/v
                            # and two q comps keep full 3-term compensation
                            lt = 1 if j < 2 else 2
                            for ti, (wi, xi) in enumerate(TERMS[:lt + 1]):
                                nc.tensor.matmul(
                                    qkv_ps[j][:],
                                    Wv[:, wi, :, j * HS:(j + 1) * HS],
                                    Xv[:, xi],
                                    start=(n == 0 and ti == 0),
                                    stop=(n == NC2 - 1 and ti == lt),
                                    perf_mode=DR)
                        # spread the previous block's rope/transpose PE
                        # work between this block's GEMM chunks so the PE
                        # queue never drains on the Act/DVE rope chain
                        if deferred and n < len(deferred):
                            deferred[n]()
                        # spread the constant loads through tb0's compute
                        # so they don't contend with x loads at the tb0/tb1
                        # boundary; cos/sin load one t-block slice per tb
                        if tb == 0 and 4 <= n < 8:
                            cdma = [(PERM, d_perm), (IDN, d_idn),
                                    (ONESB, d_ones), (TRI, d_tri)][n - 4]
                            eng = nc.gpsimd if n % 2 == 0 else nc.sync
                            eng.dma_start(cdma[0][:], cdma[1][:])
                        if n == 8:
                            nc.gpsimd.dma_start(COS[:, ts], d_cos[:, ts])
                        if n == 9:
                            nc.sync.dma_start(SIN[:, ts], d_sin[:, ts])
                        if tb == 0 and n == 10:
                            nc.sync.dma_start(ONES32[:], d_ones32[:])

                    deferred = []

                    for j in range(5):  # q0..q3, k get rope
                        raw = rawp.tile([128, 512], BF16, tag=f"raw{j}")
                        # alternate Act/DVE so the six descale copies drain
                        # in ~half the serial latency (frees PSUM banks and
                        # unblocks the deferred rope sooner)
                        if j % 2 == 0:
                            nc.scalar.mul(raw[:], qkv_ps[j][:], 1.0 / WSCALE)
                        else:
                            nc.vector.tensor_scalar_mul(
                                raw[:], qkv_ps[j][:], 1.0 / WSCALE)
                        deferred.append(rope_item(tb, j, raw))

                    # v: transpose [hs, t] -> [t, hs] chunks
                    vraw = rawp.tile([128, 512], BF16, tag="raw5")
                    nc.vector.tensor_scalar_mul(
                        vraw[:], qkv_ps[5][:], 1.0 / WSCALE)
                    for u in range(4):
                        deferred.append(v_item(tb, vraw, u))
                for it in deferred:   # flush the last block's rope
                    it()

            # ---------------- Phase B: causal attention ------------------
            with tc.tile_pool(name="wp", bufs=1) as wpp, \
                 tc.tile_pool(name="ptp", bufs=8) as ptp, \
                 tc.tile_pool(name="rcp", bufs=3) as rcp:
                WP = wpp.tile([128, 2 * QPK * C], FP8)   # [p, hl, h, c]
                for hl in range(2):
                    nc.sync.dma_start(
                        WP[:].rearrange("p (hl h c) -> p hl h c",
                                        hl=2, h=QPK)[:, hl],
                        d_wp[hl].rearrange("(h p) c -> p h c", h=QPK))
                # Y split per head-pair so the projection's first pair does
                # not falsely wait on writes for the second pair's heads
                Y8a = wpp.tile([128, 2 * 2 * T], FP8)    # [p, hl, h01, t]
                Y8b = wpp.tile([128, 2 * 2 * T], FP8)    # [p, hl, h23, t]

                with tc.tile_pool(name="psS", bufs=2, space="PSUM") as psS, \
                     tc.tile_pool(name="psY", bufs=2, space="PSUM") as psY, \
                     tc.tile_pool(name="psD", bufs=2, space="PSUM") as psD, \
                     tc.tile_pool(name="psO", bufs=2, space="PSUM") as psO, \
                     tc.tile_pool(name="accp", bufs=3) as accp, \
                     tc.tile_pool(name="outp", bufs=2) as outp:
                    WPv = WP[:].rearrange("p (hl h c) -> p hl h c",
                                          hl=2, h=QPK)
                    CTERMS = ((0, 0), (0, 1), (1, 0))   # (y_hl, w_hl)
                    pend_dacc = None
                    for b in range(NT):
                        for h in range(QPK):
                            nkt = 4 * (b + 1)
                            y_ps = psY.tile([128, 512], F32, tag="y")
                            d_ps = psD.tile([128, 512], F32, tag="d")
                            # non-diagonal (full-width) P tiles accumulate
                            # on DVE/Pool in f32; only the 4 diagonal tiles
                            # and the two closing partition-sum matmuls use
                            # the PE for the softmax denominator
                            nacc = nkt - 4
                            acc_e = acc_o = None
                            if nacc > 0:
                                acc_e = accp.tile([128, 512], F32R, tag="ae")
                                acc_o = accp.tile([128, 512], F32R, tag="ao")
                            first_e = first_o = True
                            # software pipeline: issue QK(kt) before
                            # AV/D(kt-1) so PE rolls past the exp latency
                            pend = None
                            for kt in range(nkt):
                                if kt == 2 and pend_dacc is not None:
                                    pend_dacc()
                                    pend_dacc = None
                                r = kt - 4 * b
                                # exact causal narrowing: tile r covers
                                # columns >= r*128; the leading 128-wide
                                # strip gets the triangular mask
                                off = 0 if r < 0 else r * 128
                                s_ps = psS.tile([128, 512], F32, tag="s")
                                nc.tensor.matmul(
                                    s_ps[:, off:],
                                    QQ[4][kt // 4][:, (kt % 4) * 128:
                                                   (kt % 4 + 1) * 128],
                                    QQ[h][b][:, off:],
                                    start=True, stop=True)
                                p_sb = ptp.tile([128, 512], BF16, tag="p")
                                if pend is not None:
                                    kp, offp, pp = pend
                                    nc.tensor.matmul(
                                        y_ps[:, offp:],
                                        V[kp // 4][:, (kp % 4) * 128:
                                                   (kp % 4 + 1) * 128],
                                        pp[:, offp:],
                                        start=(kp == 0), stop=False)
                                    if kp >= nacc:
                                        nc.tensor.matmul(
                                            d_ps[:, offp:], ONESB[:],
                                            pp[:, offp:],
                                            start=(kp == nacc), stop=False)
                                nc.scalar.activation(
                                    p_sb[:, off:], s_ps[:, off:],
                                    mybir.ActivationFunctionType.Exp,
                                    scale=SCALE)
                                if r >= 0:
                                    nc.vector.tensor_tensor(
                                        p_sb[:, off:off + 128],
                                        p_sb[:, off:off + 128],
                                        TRI[:],
                                        mybir.AluOpType.mult)
                                else:
                                    # full-width tile: accumulate for the
                                    # denominator off the PE (Pool is
                                    # ~1.7x slower per op -> gets 1/3)
                                    pool_turn = (kt % 3 == 2)
                                    eng = nc.gpsimd if pool_turn else nc.vector
                                    acc = acc_o if pool_turn else acc_e
                                    first = first_o if pool_turn else first_e
                                    with nc.allow_low_precision(
                                            reason="f32r acc"):
                                        if first:
                                            eng.tensor_copy(acc[:], p_sb[:])
                                        else:
                                            eng.tensor_tensor(
                                                acc[:], acc[:], p_sb[:],
                                                mybir.AluOpType.add)
                                    if pool_turn:
                                        first_o = False
                                    else:
                                        first_e = False
                                pend = (kt, off, p_sb)
                            kp, offp, pp = pend
                            nc.tensor.matmul(
                                y_ps[:, offp:],
                                V[kp // 4][:, (kp % 4) * 128:
                                           (kp % 4 + 1) * 128],
                                pp[:, offp:],
                                start=(kp == 0), stop=True)
                            nc.tensor.matmul(
                                d_ps[:, offp:], ONESB[:], pp[:, offp:],
                                start=(kp == nacc), stop=(nacc == 0))

                            def dacc_close(b=b, h=h, nacc=nacc,
                                           acc_e=(acc_e if nacc else None),
                                           acc_o=(acc_o if nacc else None),
                                           d_ps=d_ps, y_ps=y_ps):
                                if nacc > 0:
                                    nc.tensor.matmul(
                                        d_ps[:], ONES32[:], acc_e[:],
                                        start=False, stop=False)
                                    nc.tensor.matmul(
                                        d_ps[:], ONES32[:], acc_o[:],
                                        start=False, stop=True)
                                recip = rcp.tile([128, 512], F32R, tag="r")
                                with nc.allow_low_precision(
                                        reason="float32r is float32-width"):
                                    nc.vector.reciprocal(recip[:], d_ps[:])
                                y_bf = rcp.tile([128, 512], BF16, tag="ybf")
                                nc.vector.tensor_tensor(
                                    y_bf[:], y_ps[:], recip[:],
                                    mybir.AluOpType.mult)
                                bs = slice(b * 512, (b + 1) * 512)
                                Yp = (Y8a if h < 2 else Y8b)[:].rearrange(
                                    "p (hl h t) -> p hl h t", hl=2, h=2)
                                nc.scalar.copy(Yp[:, 0, h % 2, bs], y_bf[:])
                                with nc.allow_low_precision(
                                        reason="fp8 residual split"):
                                    nc.vector.tensor_tensor(
                                        Yp[:, 1, h % 2, bs], y_bf[:],
                                        Yp[:, 0, h % 2, bs],
                                        mybir.AluOpType.subtract)
                            # defer the denominator close + normalize into
                            # the next head's QK stream so the PE doesn't
                            # idle waiting for the lagging DVE/Pool adds
                            pend_dacc = dacc_close

                        if pend_dacc is not None:
                            pend_dacc()
                            pend_dacc = None

                    # ---- output projection, deferred one block ----
                    # proj(pb) is emitted after attention of block pb+1, so
                    # the next block's GEMMs hide the last head's epilogue
                    # chain; the final block's projection follows the
                    # second-to-last's with all dependencies long satisfied
                    for pb in ([b - 1, b] if b == NT - 1 else [b - 1]):
                        if pb < 0:
                            continue
                        for tt in range(4 * pb, 4 * (pb + 1)):
                            o_sb = outp.tile([128, C], BF16, tag="o")
                            for cb in range(C // 512):
                                o_ps = psO.tile([128, 512], F32, tag="o")
                                for hp in range(QPK // 2):
                                    Ypr = (Y8a if hp == 0 else Y8b)[:]\
                                        .rearrange("p (hl h t) -> p hl h t",
                                                   hl=2, h=2)
                                    hsl = slice(2 * hp, 2 * hp + 2)
                                    tsl = slice(tt * 128, (tt + 1) * 128)
                                    csl = slice(cb * 512, (cb + 1) * 512)
                                    for ti, (yi, wi) in enumerate(CTERMS):
                                        nc.tensor.matmul(
                                            o_ps[:],
                                            Ypr[:, yi, :, tsl],
                                            WPv[:, wi, hsl, csl],
                                            start=(hp == 0 and ti == 0),
                                            stop=(hp == 1 and ti == 2),
                                            perf_mode=DR)
                                # alternate Act/DVE for PSUM->SBUF copies
                                if cb % 2 == 0:
                                    nc.scalar.copy(
                                        o_sb[:, cb * 512:(cb + 1) * 512],
                                        o_ps[:])
                                else:
                                    nc.vector.tensor_copy(
                                        o_sb[:, cb * 512:(cb + 1) * 512],
                                        o_ps[:])
                                if cb % 2 == 1:
                                    cs0 = (cb - 1) * 512
                                    nc.sync.dma_start(
                                        d_out[tt * 128:(tt + 1) * 128,
                                              cs0:cs0 + 1024],
                                        o_sb[:, cs0:cs0 + 1024])
    nc.compile()
    return nc


def _split8(a):
    """fp8 hi/lo split: a ~ unit sigma -> [2, ...] e4m3 stack."""
    hi = a.astype(NP8)
    lo = (a - hi.astype(np.float32)).astype(NP8)
    return np.ascontiguousarray(np.stack([hi, lo]))


def _host_inputs(x, cos, sin, w_attn, w_proj):
    """Build per-core input maps (host-side shard + transpose prep)."""
    f = np.float32
    x8 = _split8(np.ascontiguousarray(x.reshape(T, C).T))            # [2, C, T]
    cost = np.ascontiguousarray(cos.T).astype(NPBF16)                # [HS, T]
    sgn = np.ones((HS, 1), f)
    sgn[:HS // 2] = -1.0
    sint = np.ascontiguousarray(sin.T * sgn).astype(NPBF16)          # signed sin
    # rot(x)=P@x in [d,t] layout; matmul computes lhsT.T @ rhs -> lhsT = P.T
    P = np.zeros((HS, HS), f)
    for i in range(HS // 2):
        P[i, i + HS // 2] = 1.0
        P[i + HS // 2, i] = 1.0
    perm = np.ascontiguousarray(P.T).astype(NPBF16)
    idn = np.eye(128, dtype=f).astype(NPBF16)
    ones = np.ones((128, 128), f).astype(NPBF16)
    # triangular strip mask: keep iff col >= row
    iidx = np.arange(128)
    tri = (iidx[None, :] >= iidx[:, None]).astype(f).astype(NPBF16)

    maps = []
    for g in range(N_CORES):
        wa8 = _split8(WSCALE *
                      np.ascontiguousarray(w_attn[g * RG:(g + 1) * RG, :].T))
        wp8 = _split8(WSCALE *
                      np.ascontiguousarray(w_proj[:, g * OG:(g + 1) * OG].T))
        maps.append({
            "x8": x8, "wa8": wa8, "wp8": wp8, "cost": cost, "sint": sint,
            "tri": tri, "perm": perm, "idn": idn, "ones": ones,
            "ones32": np.ones((128, 128), f),
        })
    return maps


_PROGRAM = None


def kernel(x, cos, sin, w_attn, w_proj):
    global _PROGRAM
    if _PROGRAM is None:
        _PROGRAM = _build_program()
    maps = _host_inputs(np.asarray(x), np.asarray(cos), np.asarray(sin),
                        np.asarray(w_attn), np.asarray(w_proj))
    res = bass_utils.run_bass_kernel_spmd(_PROGRAM, maps, list(range(N_CORES)))
    out = np.zeros((T, C), np.float32)
    for g in range(N_CORES):
        out += np.asarray(res.results[g]["out"], dtype=np.float32)
    return (out / WSCALE).reshape(1, T, C)


# revision 81
# speedup vs baseline: 1.2145x; 1.0168x over previous
"""Trainium2 Bass kernel for CausalSelfAttention (B=1, T=2048, C=4096,
32 heads / 8 query groups / head_size 128, full-dim RoPE, GQA).

Sharding: tensor-parallel over the 8 query groups. Core g owns w_attn rows
[g*768:(g+1)*768] (4 q heads + 1 k + 1 v) and w_proj columns
[g*512:(g+1)*512]; x is replicated. Each core returns a partial projection
output [2048, 4096] (bf16); the host sums the 8 partials (the all-reduce).

Final architecture (418534ns baseline -> 317475ns):
- qkv + output projections run as fp8e4 (e4m3) DoubleRow matmuls (0.5
  cycles/row, 2x contraction per instr) with 3-term hi/lo error
  compensation (xh@wh + xl@wh + xh@wl); weights pre-scaled x64 to escape
  e4m3's subnormal range, descaled in the PSUM-read copies / host gather.
- attention in bf16 with exact causal narrowing; scores kept transposed
  ([k, q]) so AV accumulates in PSUM; no max-subtraction needed (fp32/bf16
  exp range suffices; fp8 P is impossible: diagonal-dominant scores
  overflow e4m3).
- softmax denominator: full-width P tiles accumulate elementwise on
  DVE (2/3) + Pool (1/3) in f32; diagonal tiles and two closing all-ones
  partition-sum matmuls on PE; close + normalize deferred into the next
  head's QK stream.
- scheduling: software-pipelined QK/AV, RoPE tails deferred into the next
  block's GEMM chunks, per-block projection deferred one block so the next
  block's attention hides the epilogue, split-engine DMA issue (Pool SWDGE
  + SP HWDGE), constants spread through tb0, split output stores.
"""

import os
import sys

for _p in ("/opt/trn_rl_repo", "/root/.axon_site/_ro/trn_rl_repo"):
    if os.path.isdir(_p) and _p not in sys.path:
        sys.path.insert(0, _p)

import numpy as np
import ml_dtypes

import concourse.bass as bass
import concourse.mybir as mybir
import concourse.tile as tile
from concourse import bacc, bass_utils

N_CORES = 8
T = 2048
C = 4096
HS = 128
N_HEAD = 32
G = 8                      # query groups == cores
QPK = 4                    # q heads per group
NCOMP = QPK + 2            # q0..q3, k, v
RG = NCOMP * HS            # 768 w_attn rows per group
OG = QPK * HS              # 512 proj-input cols per group
NT = T // 512              # 4 blocks of 512 along t
NC = C // 128              # 32 contraction chunks
NQ = C // 512              # 8 contraction quads
SCALE = 1.0 / np.sqrt(float(HS))

F32 = mybir.dt.float32
F32R = mybir.dt.float32r
BF16 = mybir.dt.bfloat16
FP8 = mybir.dt.float8e4
NPBF16 = ml_dtypes.bfloat16
NP8 = ml_dtypes.float8_e4m3
DR = mybir.MatmulPerfMode.DoubleRow
NC2 = C // 256             # 16 double-row pair chunks
WSCALE = 64.0              # weight pre-scale so fp8 operands are ~unit sigma


def _build_program():
    nc = bacc.Bacc(trn_type="TRN2", target_bir_lowering=False, debug=False,
                   num_devices=N_CORES)

    d_x = nc.dram_tensor("x8", [2, C, T], FP8, kind="ExternalInput").ap()
    d_wa = nc.dram_tensor("wa8", [2, C, RG], FP8, kind="ExternalInput").ap()
    d_wp = nc.dram_tensor("wp8", [2, OG, C], FP8, kind="ExternalInput").ap()
    d_cos = nc.dram_tensor("cost", [HS, T], BF16, kind="ExternalInput").ap()
    d_sin = nc.dram_tensor("sint", [HS, T], BF16, kind="ExternalInput").ap()
    d_tri = nc.dram_tensor("tri", [128, 128], BF16, kind="ExternalInput").ap()
    d_perm = nc.dram_tensor("perm", [128, 128], BF16, kind="ExternalInput").ap()
    d_idn = nc.dram_tensor("idn", [128, 128], BF16, kind="ExternalInput").ap()
    d_ones = nc.dram_tensor("ones", [128, 128], BF16, kind="ExternalInput").ap()
    d_ones32 = nc.dram_tensor("ones32", [128, 128], F32R,
                              kind="ExternalInput").ap()
    d_out = nc.dram_tensor("out", [T, C], BF16, kind="ExternalOutput").ap()

    with tile.TileContext(nc) as tc:
        with tc.tile_pool(name="glob", bufs=1) as glob:
            # roped q0..q3 / k, one tile per (comp, t-block): [hs=128, 512]
            QQ = [[glob.tile([128, 512], BF16, name=f"qq{j}_{tb}",
                             tag=f"qq{j}_{tb}")
                   for tb in range(NT)] for j in range(5)]
            # V in [t, hs] layout, one tile per t-block: col u = t-chunk
            V = [glob.tile([128, 512], BF16, name=f"v{tb}", tag=f"v{tb}")
                 for tb in range(NT)]
            ONESB = glob.tile([128, 128], BF16)
            ONES32 = glob.tile([128, 128], F32R)
            PERM = glob.tile([128, 128], BF16)
            IDN = glob.tile([128, 128], BF16)
            TRI = glob.tile([128, 128], BF16)
            COS = glob.tile([128, T], BF16)
            SIN = glob.tile([128, T], BF16)

            # ---------------- Phase A: qkv projection + rope -------------
            with tc.tile_pool(name="wa", bufs=1) as wap, \
                 tc.tile_pool(name="xp", bufs=10) as xp, \
                 tc.tile_pool(name="rawp", bufs=2) as rawp, \
                 tc.tile_pool(name="tmpa", bufs=2) as tmpa, \
                 tc.tile_pool(name="psA", bufs=1, space="PSUM") as psA, \
                 tc.tile_pool(name="psR", bufs=2, space="PSUM") as psR:
                # weight pair-chunks for DoubleRow: [p, hl, s, r] per n2
                WA = [wap.tile([128, 2 * 2 * RG], FP8, name=f"wa{n}",
                               tag=f"wa{n}")
                      for n in range(NC2)]
                # fp8 3-term: hi@hi + hi@lo + lo@hi (w_hl, x_hl)
                TERMS = ((0, 0), (0, 1), (1, 0))

                def rope_item(tb, j, raw):
                    """PE/DVE tail of rope for comp j of block tb."""
                    ts = slice(tb * 512, (tb + 1) * 512)
                    def emit():
                        rot = psR.tile([128, 512], F32, tag="rot")
                        nc.tensor.matmul(rot[:], PERM[:], raw[:],
                                         start=True, stop=True)
                        t1 = tmpa.tile([128, 512], BF16, tag="t1")
                        nc.vector.tensor_tensor(t1[:], raw[:], COS[:, ts],
                                                mybir.AluOpType.mult)
                        t2 = tmpa.tile([128, 512], BF16, tag="t2")
                        nc.vector.tensor_tensor(t2[:], rot[:], SIN[:, ts],
                                                mybir.AluOpType.mult)
                        nc.vector.tensor_tensor(QQ[j][tb][:], t1[:], t2[:],
                                                mybir.AluOpType.add)
                    return emit

                def v_item(tb, vraw, u):
                    def emit():
                        vt = psR.tile([128, 128], BF16, tag="rot")
                        nc.tensor.transpose(
                            vt[:], vraw[:, u * 128:(u + 1) * 128], IDN[:])
                        nc.scalar.copy(V[tb][:, u * 128:(u + 1) * 128],
                                       vt[:])
                    return emit

                deferred = []
                for tb in range(NT):
                    ts = slice(tb * 512, (tb + 1) * 512)
                    qkv_ps = [psA.tile([128, 512], F32, tag=f"qkv{j}",
                                       name=f"qkv{j}")
                              for j in range(NCOMP)]
                    for n in range(NC2):
                        if tb == 0:
                            for hl in range(2):
                                nc.sync.dma_start(
                                    WA[n][:].rearrange(
                                        "p (hl s r) -> p hl s r",
                                        hl=2, s=2)[:, hl],
                                    d_wa[hl, 256 * n:256 * (n + 1), :]
                                    .rearrange("(s p) r -> p s r", s=2))
                        xt = xp.tile([128, 2 * 2 * 512], FP8, tag="x")
                        for hl in range(2):
                            # split DMA issue: hi plane on the idle Pool
                            # SWDGE queue, lo plane on SP/HWDGE, so neither
                            # generator paces the matmul loop
                            eng = nc.gpsimd if hl == 0 else nc.sync
                            eng.dma_start(
                                xt[:].rearrange("p (hl s t) -> p hl s t",
                                                hl=2, s=2)[:, hl],
                                d_x[hl, 256 * n:256 * (n + 1), ts]
                                .rearrange("(s p) t -> p s t", s=2))
                        Wv = WA[n][:].rearrange("p (hl s r) -> p hl s r",
                                                hl=2, s=2)
                        Xv = xt[:].rearrange("p (hl s t) -> p hl s t",
                                             hl=2, s=2)
                        for j in range(NCOMP):
                            # q0/q1/q2 skip the w-lo term: a q-side 2%
                            # weight quant error only perturbs one head's
                            # scores (7.5e-3/head, combining in quadrature
                            # -> ~1.2e-2 total vs the 2e-2 gate); k/v and
                            # q3 keep full 3-term compensation
                            lt = 1 if j < 3 else 2
                            for ti, (wi, xi) in enumerate(TERMS[:lt + 1]):
                                nc.tensor.matmul(
                                    qkv_ps[j][:],
                                    Wv[:, wi, :, j * HS:(j + 1) * HS],
                                    Xv[:, xi],
                                    start=(n == 0 and ti == 0),
                                    stop=(n == NC2 - 1 and ti == lt),
                                    perf_mode=DR)
                        # spread the previous block's rope/transpose PE
                        # work between this block's GEMM chunks so the PE
                        # queue never drains on the Act/DVE rope chain
                        if deferred and n < len(deferred):
                            deferred[n]()
                        # spread the constant loads through tb0's compute
                        # so they don't contend with x loads at the tb0/tb1
                        # boundary; cos/sin load one t-block slice per tb
                        if tb == 0 and 4 <= n < 8:
                            cdma = [(PERM, d_perm), (IDN, d_idn),
                                    (ONESB, d_ones), (TRI, d_tri)][n - 4]
                            eng = nc.gpsimd if n % 2 == 0 else nc.sync
                            eng.dma_start(cdma[0][:], cdma[1][:])
                        if n == 8:
                            nc.gpsimd.dma_start(COS[:, ts], d_cos[:, ts])
                        if n == 9:
                            nc.sync.dma_start(SIN[:, ts], d_sin[:, ts])
                        if tb == 0 and n == 10:
                            nc.sync.dma_start(ONES32[:], d_ones32[:])

                    deferred = []

                    for j in range(5):  # q0..q3, k get rope
                        raw = rawp.tile([128, 512], BF16, tag=f"raw{j}")
                        # alternate Act/DVE so the six descale copies drain
                        # in ~half the serial latency (frees PSUM banks and
                        # unblocks the deferred rope sooner)
                        if j % 2 == 0:
                            nc.scalar.mul(raw[:], qkv_ps[j][:], 1.0 / WSCALE)
                        else:
                            nc.vector.tensor_scalar_mul(
                                raw[:], qkv_ps[j][:], 1.0 / WSCALE)
                        deferred.append(rope_item(tb, j, raw))

                    # v: transpose [hs, t] -> [t, hs] chunks
                    vraw = rawp.tile([128, 512], BF16, tag="raw5")
                    nc.vector.tensor_scalar_mul(
                        vraw[:], qkv_ps[5][:], 1.0 / WSCALE)
                    for u in range(4):
                        deferred.append(v_item(tb, vraw, u))
                for it in deferred:   # flush the last block's rope
                    it()

            # ---------------- Phase B: causal attention ------------------
            with tc.tile_pool(name="wp", bufs=1) as wpp, \
                 tc.tile_pool(name="ptp", bufs=8) as ptp, \
                 tc.tile_pool(name="rcp", bufs=3) as rcp:
                WP = wpp.tile([128, 2 * QPK * C], FP8)   # [p, hl, h, c]
                for hl in range(2):
                    nc.sync.dma_start(
                        WP[:].rearrange("p (hl h c) -> p hl h c",
                                        hl=2, h=QPK)[:, hl],
                        d_wp[hl].rearrange("(h p) c -> p h c", h=QPK))
                # Y split per head-pair so the projection's first pair does
                # not falsely wait on writes for the second pair's heads
                Y8a = wpp.tile([128, 2 * 2 * T], FP8)    # [p, hl, h01, t]
                Y8b = wpp.tile([128, 2 * 2 * T], FP8)    # [p, hl, h23, t]

                with tc.tile_pool(name="psS", bufs=2, space="PSUM") as psS, \
                     tc.tile_pool(name="psY", bufs=2, space="PSUM") as psY, \
                     tc.tile_pool(name="psD", bufs=2, space="PSUM") as psD, \
                     tc.tile_pool(name="psO", bufs=2, space="PSUM") as psO, \
                     tc.tile_pool(name="accp", bufs=3) as accp, \
                     tc.tile_pool(name="outp", bufs=2) as outp:
                    WPv = WP[:].rearrange("p (hl h c) -> p hl h c",
                                          hl=2, h=QPK)
                    CTERMS = ((0, 0), (0, 1), (1, 0))   # (y_hl, w_hl)
                    pend_dacc = None
                    for b in range(NT):
                        for h in range(QPK):
                            nkt = 4 * (b + 1)
                            y_ps = psY.tile([128, 512], F32, tag="y")
                            d_ps = psD.tile([128, 512], F32, tag="d")
                            # non-diagonal (full-width) P tiles accumulate
                            # on DVE/Pool in f32; only the 4 diagonal tiles
                            # and the two closing partition-sum matmuls use
                            # the PE for the softmax denominator
                            nacc = nkt - 4
                            acc_e = acc_o = None
                            if nacc > 0:
                                acc_e = accp.tile([128, 512], F32R, tag="ae")
                                acc_o = accp.tile([128, 512], F32R, tag="ao")
                            first_e = first_o = True
                            # software pipeline: issue QK(kt) before
                            # AV/D(kt-1) so PE rolls past the exp latency
                            pend = None
                            for kt in range(nkt):
                                if kt == 2 and pend_dacc is not None:
                                    pend_dacc()
                                    pend_dacc = None
                                r = kt - 4 * b
                                # exact causal narrowing: tile r covers
                                # columns >= r*128; the leading 128-wide
                                # strip gets the triangular mask
                                off = 0 if r < 0 else r * 128
                                s_ps = psS.tile([128, 512], F32, tag="s")
                                nc.tensor.matmul(
                                    s_ps[:, off:],
                                    QQ[4][kt // 4][:, (kt % 4) * 128:
                                                   (kt % 4 + 1) * 128],
                                    QQ[h][b][:, off:],
                                    start=True, stop=True)
                                p_sb = ptp.tile([128, 512], BF16, tag="p")
                                if pend is not None:
                                    kp, offp, pp = pend
                                    nc.tensor.matmul(
                                        y_ps[:, offp:],
                                        V[kp // 4][:, (kp % 4) * 128:
                                                   (kp % 4 + 1) * 128],
                                        pp[:, offp:],
                                        start=(kp == 0), stop=False)
                                    if kp >= nacc:
                                        nc.tensor.matmul(
                                            d_ps[:, offp:], ONESB[:],
                                            pp[:, offp:],
                                            start=(kp == nacc), stop=False)
                                nc.scalar.activation(
                                    p_sb[:, off:], s_ps[:, off:],
                                    mybir.ActivationFunctionType.Exp,
                                    scale=SCALE)
                                if r >= 0:
                                    nc.vector.tensor_tensor(
                                        p_sb[:, off:off + 128],
                                        p_sb[:, off:off + 128],
                                        TRI[:],
                                        mybir.AluOpType.mult)
                                else:
                                    # full-width tile: accumulate for the
                                    # denominator off the PE (Pool is
                                    # ~1.7x slower per op -> gets 1/3)
                                    pool_turn = (kt % 3 == 2)
                                    eng = nc.gpsimd if pool_turn else nc.vector
                                    acc = acc_o if pool_turn else acc_e
                                    first = first_o if pool_turn else first_e
                                    with nc.allow_low_precision(
                                            reason="f32r acc"):
                                        if first:
                                            eng.tensor_copy(acc[:], p_sb[:])
                                        else:
                                            eng.tensor_tensor(
                                                acc[:], acc[:], p_sb[:],
                                                mybir.AluOpType.add)
                                    if pool_turn:
                                        first_o = False
                                    else:
                                        first_e = False
                                pend = (kt, off, p_sb)
                            kp, offp, pp = pend
                            nc.tensor.matmul(
                                y_ps[:, offp:],
                                V[kp // 4][:, (kp % 4) * 128:
                                           (kp % 4 + 1) * 128],
                                pp[:, offp:],
                                start=(kp == 0), stop=True)
                            nc.tensor.matmul(
                                d_ps[:, offp:], ONESB[:], pp[:, offp:],
                                start=(kp == nacc), stop=(nacc == 0))

                            def dacc_close(b=b, h=h, nacc=nacc,
                                           acc_e=(acc_e if nacc else None),
                                           acc_o=(acc_o if nacc else None),
                                           d_ps=d_ps, y_ps=y_ps):
                                if nacc > 0:
                                    nc.tensor.matmul(
                                        d_ps[:], ONES32[:], acc_e[:],
                                        start=False, stop=False)
                                    nc.tensor.matmul(
                                        d_ps[:], ONES32[:], acc_o[:],
                                        start=False, stop=True)
                                recip = rcp.tile([128, 512], F32R, tag="r")
                                with nc.allow_low_precision(
                                        reason="float32r is float32-width"):
                                    nc.vector.reciprocal(recip[:], d_ps[:])
                                y_bf = rcp.tile([128, 512], BF16, tag="ybf")
                                nc.vector.tensor_tensor(
                                    y_bf[:], y_ps[:], recip[:],
                                    mybir.AluOpType.mult)
                                bs = slice(b * 512, (b + 1) * 512)
                                Yp = (Y8a if h < 2 else Y8b)[:].rearrange(
                                    "p (hl h t) -> p hl h t", hl=2, h=2)
                                nc.scalar.copy(Yp[:, 0, h % 2, bs], y_bf[:])
                                with nc.allow_low_precision(
                                        reason="fp8 residual split"):
                                    nc.vector.tensor_tensor(
                                        Yp[:, 1, h % 2, bs], y_bf[:],
                                        Yp[:, 0, h % 2, bs],
                                        mybir.AluOpType.subtract)
                            # defer the denominator close + normalize into
                            # the next head's QK stream so the PE doesn't
                            # idle waiting for the lagging DVE/Pool adds
                            pend_dacc = dacc_close

                        if pend_dacc is not None:
                            pend_dacc()
                            pend_dacc = None

                    # ---- output projection, deferred one block ----
                    # proj(pb) is emitted after attention of block pb+1, so
                    # the next block's GEMMs hide the last head's epilogue
                    # chain; the final block's projection follows the
                    # second-to-last's with all dependencies long satisfied
                    for pb in ([b - 1, b] if b == NT - 1 else [b - 1]):
                        if pb < 0:
                            continue
                        for tt in range(4 * pb, 4 * (pb + 1)):
                            o_sb = outp.tile([128, C], BF16, tag="o")
                            for cb in range(C // 512):
                                o_ps = psO.tile([128, 512], F32, tag="o")
                                for hp in range(QPK // 2):
                                    Ypr = (Y8a if hp == 0 else Y8b)[:]\
                                        .rearrange("p (hl h t) -> p hl h t",
                                                   hl=2, h=2)
                                    hsl = slice(2 * hp, 2 * hp + 2)
                                    tsl = slice(tt * 128, (tt + 1) * 128)
                                    csl = slice(cb * 512, (cb + 1) * 512)
                                    for ti, (yi, wi) in enumerate(CTERMS):
                                        nc.tensor.matmul(
                                            o_ps[:],
                                            Ypr[:, yi, :, tsl],
                                            WPv[:, wi, hsl, csl],
                                            start=(hp == 0 and ti == 0),
                                            stop=(hp == 1 and ti == 2),
                                            perf_mode=DR)
                                # alternate Act/DVE for PSUM->SBUF copies
                                if cb % 2 == 0:
                                    nc.scalar.copy(
                                        o_sb[:, cb * 512:(cb + 1) * 512],
                                        o_ps[:])
                                else:
                                    nc.vector.tensor_copy(
                                        o_sb[:, cb * 512:(cb + 1) * 512],
                                        o_ps[:])
                                if cb % 2 == 1:
                                    cs0 = (cb - 1) * 512
                                    nc.sync.dma_start(
                                        d_out[tt * 128:(tt + 1) * 128,
                                              cs0:cs0 + 1024],
                                        o_sb[:, cs0:cs0 + 1024])
    nc.compile()
    return nc


def _split8(a):
    """fp8 hi/lo split: a ~ unit sigma -> [2, ...] e4m3 stack."""
    hi = a.astype(NP8)
    lo = (a - hi.astype(np.float32)).astype(NP8)
    return np.ascontiguousarray(np.stack([hi, lo]))


def _host_inputs(x, cos, sin, w_attn, w_proj):
    """Build per-core input maps (host-side shard + transpose prep)."""
    f = np.float32
    x8 = _split8(np.ascontiguousarray(x.reshape(T, C).T))            # [2, C, T]
    cost = np.ascontiguousarray(cos.T).astype(NPBF16)                # [HS, T]
    sgn = np.ones((HS, 1), f)
    sgn[:HS // 2] = -1.0
    sint = np.ascontiguousarray(sin.T * sgn).astype(NPBF16)          # signed sin
    # rot(x)=P@x in [d,t] layout; matmul computes lhsT.T @ rhs -> lhsT = P.T
    P = np.zeros((HS, HS), f)
    for i in range(HS // 2):
        P[i, i + HS // 2] = 1.0
        P[i + HS // 2, i] = 1.0
    perm = np.ascontiguousarray(P.T).astype(NPBF16)
    idn = np.eye(128, dtype=f).astype(NPBF16)
    ones = np.ones((128, 128), f).astype(NPBF16)
    # triangular strip mask: keep iff col >= row
    iidx = np.arange(128)
    tri = (iidx[None, :] >= iidx[:, None]).astype(f).astype(NPBF16)

    maps = []
    for g in range(N_CORES):
        wa8 = _split8(WSCALE *
                      np.ascontiguousarray(w_attn[g * RG:(g + 1) * RG, :].T))
        wp8 = _split8(WSCALE *
                      np.ascontiguousarray(w_proj[:, g * OG:(g + 1) * OG].T))
        maps.append({
            "x8": x8, "wa8": wa8, "wp8": wp8, "cost": cost, "sint": sint,
            "tri": tri, "perm": perm, "idn": idn, "ones": ones,
            "ones32": np.ones((128, 128), f),
        })
    return maps


_PROGRAM = None


def kernel(x, cos, sin, w_attn, w_proj):
    global _PROGRAM
    if _PROGRAM is None:
        _PROGRAM = _build_program()
    maps = _host_inputs(np.asarray(x), np.asarray(cos), np.asarray(sin),
                        np.asarray(w_attn), np.asarray(w_proj))
    res = bass_utils.run_bass_kernel_spmd(_PROGRAM, maps, list(range(N_CORES)))
    out = np.zeros((T, C), np.float32)
    for g in range(N_CORES):
        out += np.asarray(res.results[g]["out"], dtype=np.float32)
    return (out / WSCALE).reshape(1, T, C)
